# revision 11
# baseline (speedup 1.0000x reference)
"""FCOS head (nn_FCOSHead_60249801228382) Trainium2 Bass kernel.

Data-parallel over batch: 8 images -> 8 NeuronCores, conv weights replicated.
Per core, FPN levels are processed in a padded flat activation layout in SBUF;
every 3x3 conv is computed as 18 accumulating matmuls (9 spatial taps x 2
input-channel chunks of 128) into PSUM, drained by the scalar engine with
fused bias + ReLU (towers) / Exp (reg head) and bf16 cast.  Levels 2-4 are
packed into one segmented pass so they share weight streams.  The 5-channel
reg+ctr final conv col-tiles 4 spatial tiles concurrently across PE column
groups.

Self-contained: only library imports (concourse et al.), shapes hardcoded.
"""

import numpy as np
import ml_dtypes

import concourse.bacc as bacc
import concourse.mybir as mybir
import concourse.tile as tile
from concourse.bass_utils import run_bass_kernel_spmd

AF = mybir.ActivationFunctionType
F32 = mybir.dt.float32
BF16 = mybir.dt.bfloat16
BF16_NP = ml_dtypes.bfloat16

LEVELS = [(100, 152), (50, 76), (25, 38), (13, 19), (7, 10)]
B, C, NCLS, STACKED = 8, 256, 80, 4
N_CORES = 8
PSUM_FREE = 512
SEG_GROUPS = [[0], [1], [2, 3, 4]]  # levels packed per processing pass

_CACHE = {}


def _level_tiles(H, W):
    """Row-aligned PSUM tiles: (r0, nr), nr*(W+2) <= PSUM_FREE."""
    Wp = W + 2
    nrmax = min(PSUM_FREE // Wp, H)
    out = []
    r0 = 1
    while r0 <= H:
        nr = min(nrmax, H - r0 + 1)
        out.append((r0, nr))
        r0 += nr
    return out


def _segments(group):
    """[(level, H, W, flat_offset)] with +2 margin cells per segment."""
    segs = []
    off = 0
    for l in group:
        H, W = LEVELS[l]
        segs.append((l, H, W, off))
        off += (H + 2) * (W + 2) + 2
    return segs


def build_nc():
    nc = bacc.Bacc(trn_type="TRN2", num_swdge_queues=4)

    xs = [
        nc.dram_tensor(f"x{i}", [C, h, w], BF16, kind="ExternalInput")
        for i, (h, w) in enumerate(LEVELS)
    ]
    tw = nc.dram_tensor("tw", [2, STACKED, 2, 128, 9 * 256], BF16, kind="ExternalInput")
    fwc = nc.dram_tensor("fw_cls", [2, 128, 9, NCLS], BF16, kind="ExternalInput")
    fwr = nc.dram_tensor("fw_rc", [2, 128, 9, 5], BF16, kind="ExternalInput")
    tb = nc.dram_tensor("tb", [128, 16], F32, kind="ExternalInput")
    fbc = nc.dram_tensor("fb_cls", [128, 1], F32, kind="ExternalInput")
    fbr = nc.dram_tensor("fb_rc", [128, 5], F32, kind="ExternalInput")
    fsr = nc.dram_tensor("fs_rc", [128, 5], F32, kind="ExternalInput")

    ocs = [
        nc.dram_tensor(f"oc{l}", [NCLS, h, w], F32, kind="ExternalOutput")
        for l, (h, w) in enumerate(LEVELS)
    ]
    ors = [
        nc.dram_tensor(f"or{l}", [4, h, w], F32, kind="ExternalOutput")
        for l, (h, w) in enumerate(LEVELS)
    ]
    ots = [
        nc.dram_tensor(f"ot{l}", [1, h, w], F32, kind="ExternalOutput")
        for l, (h, w) in enumerate(LEVELS)
    ]

    SZ0 = max(
        sum((h + 2) * (w + 2) + 2 for _, h, w, _ in _segments(g)) for g in SEG_GROUPS
    )

    with tile.TileContext(nc) as tc:
        with (
            tc.tile_pool(name="wpool", bufs=1) as wpool,
            tc.tile_pool(name="wstream", bufs=5) as wstream,
            tc.tile_pool(name="abuf", bufs=1) as abuf,
            tc.tile_pool(name="psum", bufs=8, space="PSUM") as psum,
            tc.tile_pool(name="stage", bufs=4) as stage,
        ):
            # ---- activation buffers: P/Q ping-pong -----------------------
            bufP = [abuf.tile([128, SZ0], BF16, name=f"P{c}", tag=f"P{c}") for c in range(2)]
            bufQ = [abuf.tile([128, SZ0], BF16, name=f"Q{c}", tag=f"Q{c}") for c in range(2)]

            def memset_pads(buf, group):
                for _, H, W, off in _segments(group):
                    Wp = W + 2
                    S = (H + 2) * Wp
                    for c in range(2):
                        nc.vector.memset(buf[c][:, off : off + Wp + 2], 0.0)
                        nc.vector.memset(
                            buf[c][:, off + 1 + (H + 1) * Wp : off + S + 2], 0.0
                        )
                        colpads = buf[c][:, off + 2 * Wp : off + 2 * Wp + H * Wp]
                        nc.vector.memset(
                            colpads.rearrange("p (r w) -> p r w", w=Wp)[:, :, 0:2],
                            0.0,
                        )

            def load_x(buf, group):
                for l, H, W, off in _segments(group):
                    Wp = W + 2
                    if l == 0:
                        bands = [(i * H // 4, (i + 1) * H // 4) for i in range(4)]
                    elif H > 8:
                        hb = (H + 1) // 2
                        bands = [(0, hb), (hb, H)]
                    else:
                        bands = [(0, H)]
                    for r0, r1 in bands:
                        for c in range(2):
                            dst = buf[c][
                                :,
                                off + Wp + 2 + r0 * Wp : off + Wp + 2 + r1 * Wp,
                            ].rearrange("p (r w) -> p r w", w=Wp)[:, :, 0:W]
                            eng = nc.sync if c == 0 else nc.gpsimd
                            eng.dma_start(
                                dst, xs[l][c * 128 : (c + 1) * 128, r0:r1, :]
                            )

            def load_tw(t, j):
                w_tile = wstream.tile([128, 2, 9, 256], BF16, name="tws", tag="tws")
                for ci in range(2):
                    nc.gpsimd.dma_start(
                        w_tile[:, ci, :, :],
                        tw[t, j, ci, :, :].rearrange("p (k m) -> p k m", k=9),
                    )
                return w_tile

            def conv_tower(src, dst, t, j, group):
                """dst = relu(conv3x3(src) + b) for every segment of group."""
                bcol0 = (t * STACKED + j) * 2
                wt = load_tw(t, j)
                for _, H, W, off in _segments(group):
                    Wp = W + 2
                    for r0, nr in _level_tiles(H, W):
                        N = nr * Wp
                        base = off + 1 + r0 * Wp
                        for co in range(2):
                            ps = psum.tile([128, N], F32, name="ps", tag="ps")
                            kk = 0
                            for ci in range(2):
                                for dy in range(3):
                                    for dx in range(3):
                                        st = base + (dy - 1) * Wp + (dx - 1)
                                        nc.tensor.matmul(
                                            ps[:, 0:N],
                                            wt[:, ci, dy * 3 + dx, co * 128 : (co + 1) * 128],
                                            src[ci][:, st : st + N],
                                            start=(kk == 0),
                                            stop=(kk == 17),
                                        )
                                        kk += 1
                            ps_v = ps[:, 0:N].rearrange("p (r w) -> p r w", w=Wp)[
                                :, :, 1 : 1 + W
                            ]
                            dst_v = dst[co][:, base : base + N].rearrange(
                                "p (r w) -> p r w", w=Wp
                            )[:, :, 1 : 1 + W]
                            nc.scalar.activation(
                                dst_v,
                                ps_v,
                                AF.Relu,
                                bias=tbt[:, bcol0 + co : bcol0 + co + 1],
                            )

            def conv_final_cls(src, group):
                for l, H, W, off in _segments(group):
                    Wp = W + 2
                    oc_flat = ocs[l].ap().rearrange("c h w -> c (h w)")
                    for r0, nr in _level_tiles(H, W):
                        N = nr * Wp
                        base = off + 1 + r0 * Wp
                        ps = psum.tile([128, N], F32, name="ps", tag="ps")
                        kk = 0
                        for ci in range(2):
                            for dy in range(3):
                                for dx in range(3):
                                    st = base + (dy - 1) * Wp + (dx - 1)
                                    nc.tensor.matmul(
                                        ps[0:NCLS, 0:N],
                                        fwct[ci][:, dy * 3 + dx, :],
                                        src[ci][:, st : st + N],
                                        start=(kk == 0),
                                        stop=(kk == 17),
                                    )
                                    kk += 1
                        sg = stage.tile([128, PSUM_FREE], F32, name="sg", tag="sg")
                        ps_v = ps[0:NCLS, 0:N].rearrange("p (r w) -> p r w", w=Wp)[
                            :, :, 1 : 1 + W
                        ]
                        nc.scalar.activation(
                            sg[0:NCLS, 0 : nr * W].rearrange("p (r w) -> p r w", w=W),
                            ps_v,
                            AF.Identity,
                            bias=fbct[0:NCLS, 0:1],
                        )
                        nc.sync.dma_start(
                            oc_flat[:, (r0 - 1) * W : (r0 - 1 + nr) * W],
                            sg[0:NCLS, 0 : nr * W],
                        )

            def conv_final_rc(src, group):
                # 4 spatial tiles concurrently via PE column groups 0..3
                jobs = []
                for l, H, W, off in _segments(group):
                    for r0, nr in _level_tiles(H, W):
                        jobs.append((l, H, W, off, r0, nr))
                for q0 in range(0, len(jobs), 4):
                    quad = jobs[q0 : q0 + 4]
                    ps = psum.tile([128, PSUM_FREE], F32, name="ps", tag="ps")
                    kk = 0
                    for ci in range(2):
                        for dy in range(3):
                            for dx in range(3):
                                for g, (l, H, W, off, r0, nr) in enumerate(quad):
                                    Wp = W + 2
                                    N = nr * Wp
                                    st = (
                                        off + 1 + r0 * Wp + (dy - 1) * Wp + (dx - 1)
                                    )
                                    nc.tensor.matmul(
                                        ps[32 * g : 32 * g + 5, 0:N],
                                        fwrt[ci][:, dy * 3 + dx, :],
                                        src[ci][:, st : st + N],
                                        start=(kk == 0),
                                        stop=(kk == 17),
                                        tile_position=(0, 32 * g),
                                    )
                                kk += 1
                    sg = stage.tile([128, PSUM_FREE], F32, name="sg", tag="sg")
                    sg2 = stage.tile([128, PSUM_FREE], F32, name="sg2", tag="sg2")
                    for g, (l, H, W, off, r0, nr) in enumerate(quad):
                        Wp = W + 2
                        N = nr * Wp
                        p0 = 32 * g
                        or_flat = ors[l].ap().rearrange("c h w -> c (h w)")
                        ot_flat = ots[l].ap().rearrange("c h w -> c (h w)")
                        ps_v = ps[p0 : p0 + 5, 0:N].rearrange(
                            "p (r w) -> p r w", w=Wp
                        )[:, :, 1 : 1 + W]
                        # y = scale*conv + bias (reg rows: coef*conv + coef*b;
                        # ctr row: conv + b)
                        nc.scalar.activation(
                            sg[p0 : p0 + 5, 0 : nr * W].rearrange(
                                "p (r w) -> p r w", w=W
                            ),
                            ps_v,
                            AF.Identity,
                            bias=fbrt[p0 : p0 + 5, l : l + 1],
                            scale=fsrt[p0 : p0 + 5, l : l + 1],
                        )
                        nc.scalar.activation(
                            sg2[p0 : p0 + 4, 0 : nr * W],
                            sg[p0 : p0 + 4, 0 : nr * W],
                            AF.Exp,
                        )
                        nc.sync.dma_start(
                            or_flat[:, (r0 - 1) * W : (r0 - 1 + nr) * W],
                            sg2[p0 : p0 + 4, 0 : nr * W],
                        )
                        nc.sync.dma_start(
                            ot_flat[:, (r0 - 1) * W : (r0 - 1 + nr) * W],
                            sg[p0 + 4 : p0 + 5, 0 : nr * W],
                        )

            # ---- first input load (head of the SP DMA ring) --------------
            memset_pads(bufP, SEG_GROUPS[0])
            load_x(bufP, SEG_GROUPS[0])
            memset_pads(bufQ, SEG_GROUPS[0])

            # ---- resident final weights / biases (SWDGE; off the SP ring)
            fwct = []
            fwrt = []
            for ci in range(2):
                w_tile = wpool.tile([128, 9, NCLS], BF16, name=f"fwc{ci}", tag=f"fwc{ci}")
                nc.gpsimd.dma_start(w_tile[:], fwc[ci, :, :, :])
                fwct.append(w_tile)
                w_tile = wpool.tile([128, 9, 5], BF16, name=f"fwr{ci}", tag=f"fwr{ci}")
                nc.gpsimd.dma_start(w_tile[:], fwr[ci, :, :, :])
                fwrt.append(w_tile)
            tbt = wpool.tile([128, 16], F32, name="tb", tag="tb")
            nc.gpsimd.dma_start(tbt[:], tb[:, :])
            fbct = wpool.tile([128, 1], F32, name="fbc", tag="fbc")
            nc.gpsimd.dma_start(fbct[:], fbc[:, :])
            fbrt = wpool.tile([128, 5], F32, name="fbr", tag="fbr")
            nc.gpsimd.dma_start(fbrt[:], fbr[:, :])
            fsrt = wpool.tile([128, 5], F32, name="fsr", tag="fsr")
            nc.gpsimd.dma_start(fsrt[:], fsr[:, :])

            # ---- main schedule: P/Q ping-pong, x re-loaded for reg tower -
            for gi, group in enumerate(SEG_GROUPS):
                conv_tower(bufP, bufQ, 0, 0, group)  # c1 cls (P holds x)
                conv_tower(bufQ, bufP, 0, 1, group)  # c2 cls
                conv_tower(bufP, bufQ, 0, 2, group)  # c3 cls
                conv_tower(bufQ, bufP, 0, 3, group)  # c4 cls
                conv_final_cls(bufP, group)
                load_x(bufQ, group)                  # re-load x (Q free after c4 cls)
                conv_tower(bufQ, bufP, 1, 0, group)  # c1 reg (P free after final cls)
                conv_tower(bufP, bufQ, 1, 1, group)  # c2 reg
                conv_tower(bufQ, bufP, 1, 2, group)  # c3 reg
                conv_tower(bufP, bufQ, 1, 3, group)  # c4 reg
                if gi + 1 < len(SEG_GROUPS):
                    # P free after c4 reg read it: prep next group's input
                    memset_pads(bufP, SEG_GROUPS[gi + 1])
                    load_x(bufP, SEG_GROUPS[gi + 1])
                conv_final_rc(bufQ, group)
                if gi + 1 < len(SEG_GROUPS):
                    memset_pads(bufQ, SEG_GROUPS[gi + 1])

    nc.compile()
    return nc


def _prep_weights(inputs):
    """Host-side: transpose conv weights into lhsT layouts, cast to bf16."""
    cls_w = np.asarray(inputs["cls_w"], np.float32)
    reg_w = np.asarray(inputs["reg_w"], np.float32)
    cls_b = np.asarray(inputs["cls_b"], np.float32)
    reg_b = np.asarray(inputs["reg_b"], np.float32)
    fcls_w = np.asarray(inputs["fcls_w"], np.float32)
    fcls_b = np.asarray(inputs["fcls_b"], np.float32)
    freg_w = np.asarray(inputs["freg_w"], np.float32)
    freg_b = np.asarray(inputs["freg_b"], np.float32)
    fctr_w = np.asarray(inputs["fctr_w"], np.float32)
    fctr_b = np.asarray(inputs["fctr_b"], np.float32)
    reg_coef = np.asarray(inputs["reg_coef"], np.float32)

    tw = np.empty((2, STACKED, 2, 128, 9 * 256), BF16_NP)
    for t, warr in enumerate((cls_w, reg_w)):
        for j in range(STACKED):
            # [O, I, 3, 3] -> [k, I, O]
            A = warr[j].transpose(2, 3, 1, 0).reshape(9, C, C)
            for ci in range(2):
                tw[t, j, ci] = (
                    A[:, ci * 128 : (ci + 1) * 128, :]
                    .transpose(1, 0, 2)
                    .reshape(128, 9 * 256)
                    .astype(BF16_NP)
                )

    def final_lhsT(w):  # [O, C, 3, 3] -> [2, 128, 9, O]
        O = w.shape[0]
        A = w.transpose(2, 3, 1, 0).reshape(9, C, O)
        out = np.empty((2, 128, 9, O), BF16_NP)
        for ci in range(2):
            out[ci] = A[:, ci * 128 : (ci + 1) * 128, :].transpose(1, 0, 2)
        return out

    fw_cls = final_lhsT(fcls_w)
    fw_rc = final_lhsT(np.concatenate([freg_w, fctr_w], axis=0))

    tb = np.zeros((128, 16), np.float32)
    for t, barr in enumerate((cls_b, reg_b)):
        for j in range(STACKED):
            for co in range(2):
                tb[:, (t * STACKED + j) * 2 + co] = barr[j, co * 128 : (co + 1) * 128]
    fb_cls = np.zeros((128, 1), np.float32)
    fb_cls[0:NCLS, 0] = fcls_b
    fb_rc = np.zeros((128, 5), np.float32)
    fs_rc = np.ones((128, 5), np.float32)
    for l in range(len(LEVELS)):
        for g in range(4):
            fb_rc[32 * g : 32 * g + 4, l] = reg_coef[l] * freg_b
            fb_rc[32 * g + 4, l] = fctr_b[0]
            fs_rc[32 * g : 32 * g + 4, l] = reg_coef[l]
    return {
        "tw": tw,
        "fw_cls": fw_cls,
        "fw_rc": fw_rc,
        "tb": tb,
        "fb_cls": fb_cls,
        "fb_rc": fb_rc,
        "fs_rc": fs_rc,
    }


def kernel(**inputs):
    if "nc" not in _CACHE:
        _CACHE["nc"] = build_nc()
    nc = _CACHE["nc"]

    shared = _prep_weights(inputs)
    xs = [np.asarray(inputs[f"x{i}"], np.float32) for i in range(len(LEVELS))]
    xs_b = [x.astype(BF16_NP) for x in xs]

    in_maps = []
    for c in range(N_CORES):
        m = dict(shared)
        for i in range(len(LEVELS)):
            m[f"x{i}"] = np.ascontiguousarray(xs_b[i][c])
        in_maps.append(m)

    res = run_bass_kernel_spmd(nc, in_maps, core_ids=list(range(N_CORES)))

    cls_outs, reg_outs, ctr_outs = [], [], []
    for l in range(len(LEVELS)):
        cls_outs.append(np.stack([res.results[c][f"oc{l}"] for c in range(N_CORES)]))
        reg_outs.append(np.stack([res.results[c][f"or{l}"] for c in range(N_CORES)]))
        ctr_outs.append(np.stack([res.results[c][f"ot{l}"] for c in range(N_CORES)]))
    return tuple(cls_outs + reg_outs + ctr_outs)


# revision 12
# speedup vs baseline: 1.0033x; 1.0033x over previous
"""FCOS head (nn_FCOSHead_60249801228382) Trainium2 Bass kernel.

Data-parallel over batch: 8 images -> 8 NeuronCores, conv weights replicated.
Per core, FPN levels are processed in a padded flat activation layout in SBUF;
every 3x3 conv is computed as 18 accumulating matmuls (9 spatial taps x 2
input-channel chunks of 128) into PSUM, drained by the scalar engine with
fused bias + ReLU (towers) / Exp (reg head) and bf16 cast.  Levels 2-4 are
packed into one segmented pass so they share weight streams.  The 5-channel
reg+ctr final conv col-tiles 4 spatial tiles concurrently across PE column
groups.

Self-contained: only library imports (concourse et al.), shapes hardcoded.
"""

import numpy as np
import ml_dtypes

import concourse.bacc as bacc
import concourse.mybir as mybir
import concourse.tile as tile
from concourse.bass_utils import run_bass_kernel_spmd

AF = mybir.ActivationFunctionType
F32 = mybir.dt.float32
BF16 = mybir.dt.bfloat16
BF16_NP = ml_dtypes.bfloat16

LEVELS = [(100, 152), (50, 76), (25, 38), (13, 19), (7, 10)]
B, C, NCLS, STACKED = 8, 256, 80, 4
N_CORES = 8
PSUM_FREE = 512
SEG_GROUPS = [[0], [1], [2, 3, 4]]  # levels packed per processing pass

_CACHE = {}


def _level_tiles(H, W):
    """Row-aligned PSUM tiles: (r0, nr), nr*(W+2) <= PSUM_FREE."""
    Wp = W + 2
    nrmax = min(PSUM_FREE // Wp, H)
    out = []
    r0 = 1
    while r0 <= H:
        nr = min(nrmax, H - r0 + 1)
        out.append((r0, nr))
        r0 += nr
    return out


def _segments(group):
    """[(level, H, W, flat_offset)] with +2 margin cells per segment."""
    segs = []
    off = 0
    for l in group:
        H, W = LEVELS[l]
        segs.append((l, H, W, off))
        off += (H + 2) * (W + 2) + 2
    return segs


def build_nc():
    nc = bacc.Bacc(trn_type="TRN2", num_swdge_queues=4)

    xs = [
        nc.dram_tensor(f"x{i}", [C, h, w], BF16, kind="ExternalInput")
        for i, (h, w) in enumerate(LEVELS)
    ]
    tw = nc.dram_tensor("tw", [2, STACKED, 2, 128, 9 * 256], BF16, kind="ExternalInput")
    fwc = nc.dram_tensor("fw_cls", [2, 128, 9, NCLS], BF16, kind="ExternalInput")
    fwr = nc.dram_tensor("fw_rc", [2, 128, 9, 5], BF16, kind="ExternalInput")
    tb = nc.dram_tensor("tb", [128, 16], F32, kind="ExternalInput")
    fbc = nc.dram_tensor("fb_cls", [128, 1], F32, kind="ExternalInput")
    fbr = nc.dram_tensor("fb_rc", [128, 5], F32, kind="ExternalInput")
    fsr = nc.dram_tensor("fs_rc", [128, 5], F32, kind="ExternalInput")

    ocs = [
        nc.dram_tensor(f"oc{l}", [NCLS, h, w], F32, kind="ExternalOutput")
        for l, (h, w) in enumerate(LEVELS)
    ]
    ors = [
        nc.dram_tensor(f"or{l}", [4, h, w], F32, kind="ExternalOutput")
        for l, (h, w) in enumerate(LEVELS)
    ]
    ots = [
        nc.dram_tensor(f"ot{l}", [1, h, w], F32, kind="ExternalOutput")
        for l, (h, w) in enumerate(LEVELS)
    ]

    SZ0 = max(
        sum((h + 2) * (w + 2) + 2 for _, h, w, _ in _segments(g)) for g in SEG_GROUPS
    )

    with tile.TileContext(nc) as tc:
        with (
            tc.tile_pool(name="wpool", bufs=1) as wpool,
            tc.tile_pool(name="wstream", bufs=5) as wstream,
            tc.tile_pool(name="abuf", bufs=1) as abuf,
            tc.tile_pool(name="psum", bufs=8, space="PSUM") as psum,
            tc.tile_pool(name="stage", bufs=4) as stage,
        ):
            # ---- activation buffers: P/Q ping-pong -----------------------
            bufP = [abuf.tile([128, SZ0], BF16, name=f"P{c}", tag=f"P{c}") for c in range(2)]
            bufQ = [abuf.tile([128, SZ0], BF16, name=f"Q{c}", tag=f"Q{c}") for c in range(2)]

            def memset_pads(buf, group):
                for _, H, W, off in _segments(group):
                    Wp = W + 2
                    S = (H + 2) * Wp
                    for c in range(2):
                        nc.vector.memset(buf[c][:, off : off + Wp + 2], 0.0)
                        nc.vector.memset(
                            buf[c][:, off + 1 + (H + 1) * Wp : off + S + 2], 0.0
                        )
                        colpads = buf[c][:, off + 2 * Wp : off + 2 * Wp + H * Wp]
                        nc.vector.memset(
                            colpads.rearrange("p (r w) -> p r w", w=Wp)[:, :, 0:2],
                            0.0,
                        )

            def load_x(buf, group):
                for l, H, W, off in _segments(group):
                    Wp = W + 2
                    if l == 0:
                        bands = [(i * H // 4, (i + 1) * H // 4) for i in range(4)]
                    elif H > 8:
                        hb = (H + 1) // 2
                        bands = [(0, hb), (hb, H)]
                    else:
                        bands = [(0, H)]
                    for r0, r1 in bands:
                        for c in range(2):
                            dst = buf[c][
                                :,
                                off + Wp + 2 + r0 * Wp : off + Wp + 2 + r1 * Wp,
                            ].rearrange("p (r w) -> p r w", w=Wp)[:, :, 0:W]
                            nc.sync.dma_start(
                                dst, xs[l][c * 128 : (c + 1) * 128, r0:r1, :]
                            )

            def load_tw(t, j):
                w_tile = wstream.tile([128, 2, 9, 256], BF16, name="tws", tag="tws")
                for ci in range(2):
                    nc.gpsimd.dma_start(
                        w_tile[:, ci, :, :],
                        tw[t, j, ci, :, :].rearrange("p (k m) -> p k m", k=9),
                    )
                return w_tile

            def conv_tower(src, dst, t, j, group):
                """dst = relu(conv3x3(src) + b) for every segment of group."""
                bcol0 = (t * STACKED + j) * 2
                wt = load_tw(t, j)
                for _, H, W, off in _segments(group):
                    Wp = W + 2
                    for r0, nr in _level_tiles(H, W):
                        N = nr * Wp
                        base = off + 1 + r0 * Wp
                        for co in range(2):
                            ps = psum.tile([128, N], F32, name="ps", tag="ps")
                            kk = 0
                            for ci in range(2):
                                for dy in range(3):
                                    for dx in range(3):
                                        st = base + (dy - 1) * Wp + (dx - 1)
                                        nc.tensor.matmul(
                                            ps[:, 0:N],
                                            wt[:, ci, dy * 3 + dx, co * 128 : (co + 1) * 128],
                                            src[ci][:, st : st + N],
                                            start=(kk == 0),
                                            stop=(kk == 17),
                                        )
                                        kk += 1
                            ps_v = ps[:, 0:N].rearrange("p (r w) -> p r w", w=Wp)[
                                :, :, 1 : 1 + W
                            ]
                            dst_v = dst[co][:, base : base + N].rearrange(
                                "p (r w) -> p r w", w=Wp
                            )[:, :, 1 : 1 + W]
                            nc.scalar.activation(
                                dst_v,
                                ps_v,
                                AF.Relu,
                                bias=tbt[:, bcol0 + co : bcol0 + co + 1],
                            )

            def conv_final_cls(src, group):
                for l, H, W, off in _segments(group):
                    Wp = W + 2
                    oc_flat = ocs[l].ap().rearrange("c h w -> c (h w)")
                    for r0, nr in _level_tiles(H, W):
                        N = nr * Wp
                        base = off + 1 + r0 * Wp
                        ps = psum.tile([128, N], F32, name="ps", tag="ps")
                        kk = 0
                        for ci in range(2):
                            for dy in range(3):
                                for dx in range(3):
                                    st = base + (dy - 1) * Wp + (dx - 1)
                                    nc.tensor.matmul(
                                        ps[0:NCLS, 0:N],
                                        fwct[ci][:, dy * 3 + dx, :],
                                        src[ci][:, st : st + N],
                                        start=(kk == 0),
                                        stop=(kk == 17),
                                    )
                                    kk += 1
                        sg = stage.tile([128, PSUM_FREE], F32, name="sg", tag="sg")
                        ps_v = ps[0:NCLS, 0:N].rearrange("p (r w) -> p r w", w=Wp)[
                            :, :, 1 : 1 + W
                        ]
                        nc.scalar.activation(
                            sg[0:NCLS, 0 : nr * W].rearrange("p (r w) -> p r w", w=W),
                            ps_v,
                            AF.Identity,
                            bias=fbct[0:NCLS, 0:1],
                        )
                        nc.sync.dma_start(
                            oc_flat[:, (r0 - 1) * W : (r0 - 1 + nr) * W],
                            sg[0:NCLS, 0 : nr * W],
                        )

            def conv_final_rc(src, group):
                # 4 spatial tiles concurrently via PE column groups 0..3
                jobs = []
                for l, H, W, off in _segments(group):
                    for r0, nr in _level_tiles(H, W):
                        jobs.append((l, H, W, off, r0, nr))
                for q0 in range(0, len(jobs), 4):
                    quad = jobs[q0 : q0 + 4]
                    ps = psum.tile([128, PSUM_FREE], F32, name="ps", tag="ps")
                    kk = 0
                    for ci in range(2):
                        for dy in range(3):
                            for dx in range(3):
                                for g, (l, H, W, off, r0, nr) in enumerate(quad):
                                    Wp = W + 2
                                    N = nr * Wp
                                    st = (
                                        off + 1 + r0 * Wp + (dy - 1) * Wp + (dx - 1)
                                    )
                                    nc.tensor.matmul(
                                        ps[32 * g : 32 * g + 5, 0:N],
                                        fwrt[ci][:, dy * 3 + dx, :],
                                        src[ci][:, st : st + N],
                                        start=(kk == 0),
                                        stop=(kk == 17),
                                        tile_position=(0, 32 * g),
                                    )
                                kk += 1
                    sg = stage.tile([128, PSUM_FREE], F32, name="sg", tag="sg")
                    sg2 = stage.tile([128, PSUM_FREE], F32, name="sg2", tag="sg2")
                    for g, (l, H, W, off, r0, nr) in enumerate(quad):
                        Wp = W + 2
                        N = nr * Wp
                        p0 = 32 * g
                        or_flat = ors[l].ap().rearrange("c h w -> c (h w)")
                        ot_flat = ots[l].ap().rearrange("c h w -> c (h w)")
                        ps_v = ps[p0 : p0 + 5, 0:N].rearrange(
                            "p (r w) -> p r w", w=Wp
                        )[:, :, 1 : 1 + W]
                        # y = scale*conv + bias (reg rows: coef*conv + coef*b;
                        # ctr row: conv + b)
                        nc.scalar.activation(
                            sg[p0 : p0 + 5, 0 : nr * W].rearrange(
                                "p (r w) -> p r w", w=W
                            ),
                            ps_v,
                            AF.Identity,
                            bias=fbrt[p0 : p0 + 5, l : l + 1],
                            scale=fsrt[p0 : p0 + 5, l : l + 1],
                        )
                        nc.scalar.activation(
                            sg2[p0 : p0 + 4, 0 : nr * W],
                            sg[p0 : p0 + 4, 0 : nr * W],
                            AF.Exp,
                        )
                        nc.sync.dma_start(
                            or_flat[:, (r0 - 1) * W : (r0 - 1 + nr) * W],
                            sg2[p0 : p0 + 4, 0 : nr * W],
                        )
                        nc.sync.dma_start(
                            ot_flat[:, (r0 - 1) * W : (r0 - 1 + nr) * W],
                            sg[p0 + 4 : p0 + 5, 0 : nr * W],
                        )

            # ---- first input load (head of the SP DMA ring) --------------
            memset_pads(bufP, SEG_GROUPS[0])
            load_x(bufP, SEG_GROUPS[0])
            memset_pads(bufQ, SEG_GROUPS[0])

            # ---- resident final weights / biases (SWDGE; off the SP ring)
            fwct = []
            fwrt = []
            for ci in range(2):
                w_tile = wpool.tile([128, 9, NCLS], BF16, name=f"fwc{ci}", tag=f"fwc{ci}")
                nc.gpsimd.dma_start(w_tile[:], fwc[ci, :, :, :])
                fwct.append(w_tile)
                w_tile = wpool.tile([128, 9, 5], BF16, name=f"fwr{ci}", tag=f"fwr{ci}")
                nc.gpsimd.dma_start(w_tile[:], fwr[ci, :, :, :])
                fwrt.append(w_tile)
            tbt = wpool.tile([128, 16], F32, name="tb", tag="tb")
            nc.gpsimd.dma_start(tbt[:], tb[:, :])
            fbct = wpool.tile([128, 1], F32, name="fbc", tag="fbc")
            nc.gpsimd.dma_start(fbct[:], fbc[:, :])
            fbrt = wpool.tile([128, 5], F32, name="fbr", tag="fbr")
            nc.gpsimd.dma_start(fbrt[:], fbr[:, :])
            fsrt = wpool.tile([128, 5], F32, name="fsr", tag="fsr")
            nc.gpsimd.dma_start(fsrt[:], fsr[:, :])

            # ---- main schedule: P/Q ping-pong, x re-loaded for reg tower -
            for gi, group in enumerate(SEG_GROUPS):
                conv_tower(bufP, bufQ, 0, 0, group)  # c1 cls (P holds x)
                conv_tower(bufQ, bufP, 0, 1, group)  # c2 cls
                conv_tower(bufP, bufQ, 0, 2, group)  # c3 cls
                conv_tower(bufQ, bufP, 0, 3, group)  # c4 cls
                conv_final_cls(bufP, group)
                load_x(bufQ, group)                  # re-load x (Q free after c4 cls)
                conv_tower(bufQ, bufP, 1, 0, group)  # c1 reg (P free after final cls)
                conv_tower(bufP, bufQ, 1, 1, group)  # c2 reg
                conv_tower(bufQ, bufP, 1, 2, group)  # c3 reg
                conv_tower(bufP, bufQ, 1, 3, group)  # c4 reg
                if gi + 1 < len(SEG_GROUPS):
                    # P free after c4 reg read it: prep next group's input
                    memset_pads(bufP, SEG_GROUPS[gi + 1])
                    load_x(bufP, SEG_GROUPS[gi + 1])
                conv_final_rc(bufQ, group)
                if gi + 1 < len(SEG_GROUPS):
                    memset_pads(bufQ, SEG_GROUPS[gi + 1])

    nc.compile()
    return nc


def _prep_weights(inputs):
    """Host-side: transpose conv weights into lhsT layouts, cast to bf16."""
    cls_w = np.asarray(inputs["cls_w"], np.float32)
    reg_w = np.asarray(inputs["reg_w"], np.float32)
    cls_b = np.asarray(inputs["cls_b"], np.float32)
    reg_b = np.asarray(inputs["reg_b"], np.float32)
    fcls_w = np.asarray(inputs["fcls_w"], np.float32)
    fcls_b = np.asarray(inputs["fcls_b"], np.float32)
    freg_w = np.asarray(inputs["freg_w"], np.float32)
    freg_b = np.asarray(inputs["freg_b"], np.float32)
    fctr_w = np.asarray(inputs["fctr_w"], np.float32)
    fctr_b = np.asarray(inputs["fctr_b"], np.float32)
    reg_coef = np.asarray(inputs["reg_coef"], np.float32)

    tw = np.empty((2, STACKED, 2, 128, 9 * 256), BF16_NP)
    for t, warr in enumerate((cls_w, reg_w)):
        for j in range(STACKED):
            # [O, I, 3, 3] -> [k, I, O]
            A = warr[j].transpose(2, 3, 1, 0).reshape(9, C, C)
            for ci in range(2):
                tw[t, j, ci] = (
                    A[:, ci * 128 : (ci + 1) * 128, :]
                    .transpose(1, 0, 2)
                    .reshape(128, 9 * 256)
                    .astype(BF16_NP)
                )

    def final_lhsT(w):  # [O, C, 3, 3] -> [2, 128, 9, O]
        O = w.shape[0]
        A = w.transpose(2, 3, 1, 0).reshape(9, C, O)
        out = np.empty((2, 128, 9, O), BF16_NP)
        for ci in range(2):
            out[ci] = A[:, ci * 128 : (ci + 1) * 128, :].transpose(1, 0, 2)
        return out

    fw_cls = final_lhsT(fcls_w)
    fw_rc = final_lhsT(np.concatenate([freg_w, fctr_w], axis=0))

    tb = np.zeros((128, 16), np.float32)
    for t, barr in enumerate((cls_b, reg_b)):
        for j in range(STACKED):
            for co in range(2):
                tb[:, (t * STACKED + j) * 2 + co] = barr[j, co * 128 : (co + 1) * 128]
    fb_cls = np.zeros((128, 1), np.float32)
    fb_cls[0:NCLS, 0] = fcls_b
    fb_rc = np.zeros((128, 5), np.float32)
    fs_rc = np.ones((128, 5), np.float32)
    for l in range(len(LEVELS)):
        for g in range(4):
            fb_rc[32 * g : 32 * g + 4, l] = reg_coef[l] * freg_b
            fb_rc[32 * g + 4, l] = fctr_b[0]
            fs_rc[32 * g : 32 * g + 4, l] = reg_coef[l]
    return {
        "tw": tw,
        "fw_cls": fw_cls,
        "fw_rc": fw_rc,
        "tb": tb,
        "fb_cls": fb_cls,
        "fb_rc": fb_rc,
        "fs_rc": fs_rc,
    }


def kernel(**inputs):
    if "nc" not in _CACHE:
        _CACHE["nc"] = build_nc()
    nc = _CACHE["nc"]

    shared = _prep_weights(inputs)
    xs = [np.asarray(inputs[f"x{i}"], np.float32) for i in range(len(LEVELS))]
    xs_b = [x.astype(BF16_NP) for x in xs]

    in_maps = []
    for c in range(N_CORES):
        m = dict(shared)
        for i in range(len(LEVELS)):
            m[f"x{i}"] = np.ascontiguousarray(xs_b[i][c])
        in_maps.append(m)

    res = run_bass_kernel_spmd(nc, in_maps, core_ids=list(range(N_CORES)))

    cls_outs, reg_outs, ctr_outs = [], [], []
    for l in range(len(LEVELS)):
        cls_outs.append(np.stack([res.results[c][f"oc{l}"] for c in range(N_CORES)]))
        reg_outs.append(np.stack([res.results[c][f"or{l}"] for c in range(N_CORES)]))
        ctr_outs.append(np.stack([res.results[c][f"ot{l}"] for c in range(N_CORES)]))
    return tuple(cls_outs + reg_outs + ctr_outs)


# revision 13
# speedup vs baseline: 1.0067x; 1.0034x over previous
"""FCOS head (nn_FCOSHead_60249801228382) Trainium2 Bass kernel.

Data-parallel over batch: 8 images -> 8 NeuronCores, conv weights replicated.
Per core, FPN levels are processed in a padded flat activation layout in SBUF;
every 3x3 conv is computed as 18 accumulating matmuls (9 spatial taps x 2
input-channel chunks of 128) into PSUM, drained by the scalar engine with
fused bias + ReLU (towers) / Exp (reg head) and bf16 cast.  Levels 2-4 are
packed into one segmented pass so they share weight streams.  The 5-channel
reg+ctr final conv col-tiles 4 spatial tiles concurrently across PE column
groups.

Self-contained: only library imports (concourse et al.), shapes hardcoded.
"""

import numpy as np
import ml_dtypes

import concourse.bacc as bacc
import concourse.mybir as mybir
import concourse.tile as tile
from concourse.bass_utils import run_bass_kernel_spmd

AF = mybir.ActivationFunctionType
F32 = mybir.dt.float32
BF16 = mybir.dt.bfloat16
BF16_NP = ml_dtypes.bfloat16

LEVELS = [(100, 152), (50, 76), (25, 38), (13, 19), (7, 10)]
B, C, NCLS, STACKED = 8, 256, 80, 4
N_CORES = 8
PSUM_FREE = 512
SEG_GROUPS = [[0], [1], [2, 3, 4]]  # levels packed per processing pass

_CACHE = {}


def _level_tiles(H, W):
    """Row-aligned PSUM tiles: (r0, nr), nr*(W+2) <= PSUM_FREE."""
    Wp = W + 2
    nrmax = min(PSUM_FREE // Wp, H)
    out = []
    r0 = 1
    while r0 <= H:
        nr = min(nrmax, H - r0 + 1)
        out.append((r0, nr))
        r0 += nr
    return out


def _segments(group):
    """[(level, H, W, flat_offset)] with +2 margin cells per segment."""
    segs = []
    off = 0
    for l in group:
        H, W = LEVELS[l]
        segs.append((l, H, W, off))
        off += (H + 2) * (W + 2) + 2
    return segs


def build_nc():
    nc = bacc.Bacc(trn_type="TRN2", num_swdge_queues=4)

    xs = [
        nc.dram_tensor(f"x{i}", [C, h, w], BF16, kind="ExternalInput")
        for i, (h, w) in enumerate(LEVELS)
    ]
    tw = nc.dram_tensor("tw", [2, STACKED, 2, 128, 9 * 256], BF16, kind="ExternalInput")
    fwc = nc.dram_tensor("fw_cls", [2, 128, 9, NCLS], BF16, kind="ExternalInput")
    fwr = nc.dram_tensor("fw_rc", [2, 128, 9, 5], BF16, kind="ExternalInput")
    tb = nc.dram_tensor("tb", [128, 16], F32, kind="ExternalInput")
    fbc = nc.dram_tensor("fb_cls", [128, 1], F32, kind="ExternalInput")
    fbr = nc.dram_tensor("fb_rc", [128, 5], F32, kind="ExternalInput")
    fsr = nc.dram_tensor("fs_rc", [128, 5], F32, kind="ExternalInput")

    ocs = [
        nc.dram_tensor(f"oc{l}", [NCLS, h, w], F32, kind="ExternalOutput")
        for l, (h, w) in enumerate(LEVELS)
    ]
    ors = [
        nc.dram_tensor(f"or{l}", [4, h, w], F32, kind="ExternalOutput")
        for l, (h, w) in enumerate(LEVELS)
    ]
    ots = [
        nc.dram_tensor(f"ot{l}", [1, h, w], F32, kind="ExternalOutput")
        for l, (h, w) in enumerate(LEVELS)
    ]

    SZ0 = max(
        sum((h + 2) * (w + 2) + 2 for _, h, w, _ in _segments(g)) for g in SEG_GROUPS
    )

    with tile.TileContext(nc) as tc:
        with (
            tc.tile_pool(name="wpool", bufs=1) as wpool,
            tc.tile_pool(name="wstream", bufs=5) as wstream,
            tc.tile_pool(name="abuf", bufs=1) as abuf,
            tc.tile_pool(name="psum", bufs=8, space="PSUM") as psum,
            tc.tile_pool(name="stage", bufs=4) as stage,
        ):
            # ---- activation buffers: P/Q ping-pong -----------------------
            bufP = [abuf.tile([128, SZ0], BF16, name=f"P{c}", tag=f"P{c}") for c in range(2)]
            bufQ = [abuf.tile([128, SZ0], BF16, name=f"Q{c}", tag=f"Q{c}") for c in range(2)]

            def memset_pads(buf, group):
                for _, H, W, off in _segments(group):
                    Wp = W + 2
                    S = (H + 2) * Wp
                    for c in range(2):
                        nc.vector.memset(buf[c][:, off : off + Wp + 2], 0.0)
                        nc.vector.memset(
                            buf[c][:, off + 1 + (H + 1) * Wp : off + S + 2], 0.0
                        )
                        colpads = buf[c][:, off + 2 * Wp : off + 2 * Wp + H * Wp]
                        nc.vector.memset(
                            colpads.rearrange("p (r w) -> p r w", w=Wp)[:, :, 0:2],
                            0.0,
                        )

            def load_x(buf, group):
                for l, H, W, off in _segments(group):
                    Wp = W + 2
                    if l == 0:
                        bands = [(0, 8), (8, 33), (33, 58), (58, 79), (79, H)]
                    elif H > 8:
                        hb = (H + 1) // 2
                        bands = [(0, hb), (hb, H)]
                    else:
                        bands = [(0, H)]
                    for r0, r1 in bands:
                        for c in range(2):
                            dst = buf[c][
                                :,
                                off + Wp + 2 + r0 * Wp : off + Wp + 2 + r1 * Wp,
                            ].rearrange("p (r w) -> p r w", w=Wp)[:, :, 0:W]
                            nc.sync.dma_start(
                                dst, xs[l][c * 128 : (c + 1) * 128, r0:r1, :]
                            )

            def load_tw(t, j):
                w_tile = wstream.tile([128, 2, 9, 256], BF16, name="tws", tag="tws")
                for ci in range(2):
                    nc.gpsimd.dma_start(
                        w_tile[:, ci, :, :],
                        tw[t, j, ci, :, :].rearrange("p (k m) -> p k m", k=9),
                    )
                return w_tile

            def conv_tower(src, dst, t, j, group):
                """dst = relu(conv3x3(src) + b) for every segment of group."""
                bcol0 = (t * STACKED + j) * 2
                wt = load_tw(t, j)
                for _, H, W, off in _segments(group):
                    Wp = W + 2
                    for r0, nr in _level_tiles(H, W):
                        N = nr * Wp
                        base = off + 1 + r0 * Wp
                        for co in range(2):
                            ps = psum.tile([128, N], F32, name="ps", tag="ps")
                            kk = 0
                            for ci in range(2):
                                for dy in range(3):
                                    for dx in range(3):
                                        st = base + (dy - 1) * Wp + (dx - 1)
                                        nc.tensor.matmul(
                                            ps[:, 0:N],
                                            wt[:, ci, dy * 3 + dx, co * 128 : (co + 1) * 128],
                                            src[ci][:, st : st + N],
                                            start=(kk == 0),
                                            stop=(kk == 17),
                                        )
                                        kk += 1
                            ps_v = ps[:, 0:N].rearrange("p (r w) -> p r w", w=Wp)[
                                :, :, 1 : 1 + W
                            ]
                            dst_v = dst[co][:, base : base + N].rearrange(
                                "p (r w) -> p r w", w=Wp
                            )[:, :, 1 : 1 + W]
                            nc.scalar.activation(
                                dst_v,
                                ps_v,
                                AF.Relu,
                                bias=tbt[:, bcol0 + co : bcol0 + co + 1],
                            )

            def conv_final_cls(src, group):
                for l, H, W, off in _segments(group):
                    Wp = W + 2
                    oc_flat = ocs[l].ap().rearrange("c h w -> c (h w)")
                    for r0, nr in _level_tiles(H, W):
                        N = nr * Wp
                        base = off + 1 + r0 * Wp
                        ps = psum.tile([128, N], F32, name="ps", tag="ps")
                        kk = 0
                        for ci in range(2):
                            for dy in range(3):
                                for dx in range(3):
                                    st = base + (dy - 1) * Wp + (dx - 1)
                                    nc.tensor.matmul(
                                        ps[0:NCLS, 0:N],
                                        fwct[ci][:, dy * 3 + dx, :],
                                        src[ci][:, st : st + N],
                                        start=(kk == 0),
                                        stop=(kk == 17),
                                    )
                                    kk += 1
                        sg = stage.tile([128, PSUM_FREE], F32, name="sg", tag="sg")
                        ps_v = ps[0:NCLS, 0:N].rearrange("p (r w) -> p r w", w=Wp)[
                            :, :, 1 : 1 + W
                        ]
                        nc.scalar.activation(
                            sg[0:NCLS, 0 : nr * W].rearrange("p (r w) -> p r w", w=W),
                            ps_v,
                            AF.Identity,
                            bias=fbct[0:NCLS, 0:1],
                        )
                        nc.sync.dma_start(
                            oc_flat[:, (r0 - 1) * W : (r0 - 1 + nr) * W],
                            sg[0:NCLS, 0 : nr * W],
                        )

            def conv_final_rc(src, group):
                # 4 spatial tiles concurrently via PE column groups 0..3
                jobs = []
                for l, H, W, off in _segments(group):
                    for r0, nr in _level_tiles(H, W):
                        jobs.append((l, H, W, off, r0, nr))
                for q0 in range(0, len(jobs), 4):
                    quad = jobs[q0 : q0 + 4]
                    ps = psum.tile([128, PSUM_FREE], F32, name="ps", tag="ps")
                    kk = 0
                    for ci in range(2):
                        for dy in range(3):
                            for dx in range(3):
                                for g, (l, H, W, off, r0, nr) in enumerate(quad):
                                    Wp = W + 2
                                    N = nr * Wp
                                    st = (
                                        off + 1 + r0 * Wp + (dy - 1) * Wp + (dx - 1)
                                    )
                                    nc.tensor.matmul(
                                        ps[32 * g : 32 * g + 5, 0:N],
                                        fwrt[ci][:, dy * 3 + dx, :],
                                        src[ci][:, st : st + N],
                                        start=(kk == 0),
                                        stop=(kk == 17),
                                        tile_position=(0, 32 * g),
                                    )
                                kk += 1
                    sg = stage.tile([128, PSUM_FREE], F32, name="sg", tag="sg")
                    sg2 = stage.tile([128, PSUM_FREE], F32, name="sg2", tag="sg2")
                    for g, (l, H, W, off, r0, nr) in enumerate(quad):
                        Wp = W + 2
                        N = nr * Wp
                        p0 = 32 * g
                        or_flat = ors[l].ap().rearrange("c h w -> c (h w)")
                        ot_flat = ots[l].ap().rearrange("c h w -> c (h w)")
                        ps_v = ps[p0 : p0 + 5, 0:N].rearrange(
                            "p (r w) -> p r w", w=Wp
                        )[:, :, 1 : 1 + W]
                        # y = scale*conv + bias (reg rows: coef*conv + coef*b;
                        # ctr row: conv + b)
                        nc.scalar.activation(
                            sg[p0 : p0 + 5, 0 : nr * W].rearrange(
                                "p (r w) -> p r w", w=W
                            ),
                            ps_v,
                            AF.Identity,
                            bias=fbrt[p0 : p0 + 5, l : l + 1],
                            scale=fsrt[p0 : p0 + 5, l : l + 1],
                        )
                        nc.scalar.activation(
                            sg2[p0 : p0 + 4, 0 : nr * W],
                            sg[p0 : p0 + 4, 0 : nr * W],
                            AF.Exp,
                        )
                        nc.sync.dma_start(
                            or_flat[:, (r0 - 1) * W : (r0 - 1 + nr) * W],
                            sg2[p0 : p0 + 4, 0 : nr * W],
                        )
                        nc.sync.dma_start(
                            ot_flat[:, (r0 - 1) * W : (r0 - 1 + nr) * W],
                            sg[p0 + 4 : p0 + 5, 0 : nr * W],
                        )

            # ---- first input load (head of the SP DMA ring) --------------
            memset_pads(bufP, SEG_GROUPS[0])
            load_x(bufP, SEG_GROUPS[0])
            memset_pads(bufQ, SEG_GROUPS[0])

            # ---- resident final weights / biases (SWDGE; off the SP ring)
            fwct = []
            fwrt = []
            for ci in range(2):
                w_tile = wpool.tile([128, 9, NCLS], BF16, name=f"fwc{ci}", tag=f"fwc{ci}")
                nc.gpsimd.dma_start(w_tile[:], fwc[ci, :, :, :])
                fwct.append(w_tile)
                w_tile = wpool.tile([128, 9, 5], BF16, name=f"fwr{ci}", tag=f"fwr{ci}")
                nc.gpsimd.dma_start(w_tile[:], fwr[ci, :, :, :])
                fwrt.append(w_tile)
            tbt = wpool.tile([128, 16], F32, name="tb", tag="tb")
            nc.gpsimd.dma_start(tbt[:], tb[:, :])
            fbct = wpool.tile([128, 1], F32, name="fbc", tag="fbc")
            nc.gpsimd.dma_start(fbct[:], fbc[:, :])
            fbrt = wpool.tile([128, 5], F32, name="fbr", tag="fbr")
            nc.gpsimd.dma_start(fbrt[:], fbr[:, :])
            fsrt = wpool.tile([128, 5], F32, name="fsr", tag="fsr")
            nc.gpsimd.dma_start(fsrt[:], fsr[:, :])

            # ---- main schedule: P/Q ping-pong, x re-loaded for reg tower -
            for gi, group in enumerate(SEG_GROUPS):
                conv_tower(bufP, bufQ, 0, 0, group)  # c1 cls (P holds x)
                conv_tower(bufQ, bufP, 0, 1, group)  # c2 cls
                conv_tower(bufP, bufQ, 0, 2, group)  # c3 cls
                conv_tower(bufQ, bufP, 0, 3, group)  # c4 cls
                conv_final_cls(bufP, group)
                load_x(bufQ, group)                  # re-load x (Q free after c4 cls)
                conv_tower(bufQ, bufP, 1, 0, group)  # c1 reg (P free after final cls)
                conv_tower(bufP, bufQ, 1, 1, group)  # c2 reg
                conv_tower(bufQ, bufP, 1, 2, group)  # c3 reg
                conv_tower(bufP, bufQ, 1, 3, group)  # c4 reg
                if gi + 1 < len(SEG_GROUPS):
                    # P free after c4 reg read it: prep next group's input
                    memset_pads(bufP, SEG_GROUPS[gi + 1])
                    load_x(bufP, SEG_GROUPS[gi + 1])
                conv_final_rc(bufQ, group)
                if gi + 1 < len(SEG_GROUPS):
                    memset_pads(bufQ, SEG_GROUPS[gi + 1])

    nc.compile()
    return nc


def _prep_weights(inputs):
    """Host-side: transpose conv weights into lhsT layouts, cast to bf16."""
    cls_w = np.asarray(inputs["cls_w"], np.float32)
    reg_w = np.asarray(inputs["reg_w"], np.float32)
    cls_b = np.asarray(inputs["cls_b"], np.float32)
    reg_b = np.asarray(inputs["reg_b"], np.float32)
    fcls_w = np.asarray(inputs["fcls_w"], np.float32)
    fcls_b = np.asarray(inputs["fcls_b"], np.float32)
    freg_w = np.asarray(inputs["freg_w"], np.float32)
    freg_b = np.asarray(inputs["freg_b"], np.float32)
    fctr_w = np.asarray(inputs["fctr_w"], np.float32)
    fctr_b = np.asarray(inputs["fctr_b"], np.float32)
    reg_coef = np.asarray(inputs["reg_coef"], np.float32)

    tw = np.empty((2, STACKED, 2, 128, 9 * 256), BF16_NP)
    for t, warr in enumerate((cls_w, reg_w)):
        for j in range(STACKED):
            # [O, I, 3, 3] -> [k, I, O]
            A = warr[j].transpose(2, 3, 1, 0).reshape(9, C, C)
            for ci in range(2):
                tw[t, j, ci] = (
                    A[:, ci * 128 : (ci + 1) * 128, :]
                    .transpose(1, 0, 2)
                    .reshape(128, 9 * 256)
                    .astype(BF16_NP)
                )

    def final_lhsT(w):  # [O, C, 3, 3] -> [2, 128, 9, O]
        O = w.shape[0]
        A = w.transpose(2, 3, 1, 0).reshape(9, C, O)
        out = np.empty((2, 128, 9, O), BF16_NP)
        for ci in range(2):
            out[ci] = A[:, ci * 128 : (ci + 1) * 128, :].transpose(1, 0, 2)
        return out

    fw_cls = final_lhsT(fcls_w)
    fw_rc = final_lhsT(np.concatenate([freg_w, fctr_w], axis=0))

    tb = np.zeros((128, 16), np.float32)
    for t, barr in enumerate((cls_b, reg_b)):
        for j in range(STACKED):
            for co in range(2):
                tb[:, (t * STACKED + j) * 2 + co] = barr[j, co * 128 : (co + 1) * 128]
    fb_cls = np.zeros((128, 1), np.float32)
    fb_cls[0:NCLS, 0] = fcls_b
    fb_rc = np.zeros((128, 5), np.float32)
    fs_rc = np.ones((128, 5), np.float32)
    for l in range(len(LEVELS)):
        for g in range(4):
            fb_rc[32 * g : 32 * g + 4, l] = reg_coef[l] * freg_b
            fb_rc[32 * g + 4, l] = fctr_b[0]
            fs_rc[32 * g : 32 * g + 4, l] = reg_coef[l]
    return {
        "tw": tw,
        "fw_cls": fw_cls,
        "fw_rc": fw_rc,
        "tb": tb,
        "fb_cls": fb_cls,
        "fb_rc": fb_rc,
        "fs_rc": fs_rc,
    }


def kernel(**inputs):
    if "nc" not in _CACHE:
        _CACHE["nc"] = build_nc()
    nc = _CACHE["nc"]

    shared = _prep_weights(inputs)
    xs = [np.asarray(inputs[f"x{i}"], np.float32) for i in range(len(LEVELS))]
    xs_b = [x.astype(BF16_NP) for x in xs]

    in_maps = []
    for c in range(N_CORES):
        m = dict(shared)
        for i in range(len(LEVELS)):
            m[f"x{i}"] = np.ascontiguousarray(xs_b[i][c])
        in_maps.append(m)

    res = run_bass_kernel_spmd(nc, in_maps, core_ids=list(range(N_CORES)))

    cls_outs, reg_outs, ctr_outs = [], [], []
    for l in range(len(LEVELS)):
        cls_outs.append(np.stack([res.results[c][f"oc{l}"] for c in range(N_CORES)]))
        reg_outs.append(np.stack([res.results[c][f"or{l}"] for c in range(N_CORES)]))
        ctr_outs.append(np.stack([res.results[c][f"ot{l}"] for c in range(N_CORES)]))
    return tuple(cls_outs + reg_outs + ctr_outs)


# revision 14
# speedup vs baseline: 1.0192x; 1.0124x over previous
"""FCOS head (nn_FCOSHead_60249801228382) Trainium2 Bass kernel.

Data-parallel over batch: 8 images -> 8 NeuronCores, conv weights replicated.
Per core, FPN levels are processed in a padded flat activation layout in SBUF;
every 3x3 conv is computed as 18 accumulating matmuls (9 spatial taps x 2
input-channel chunks of 128) into PSUM, drained by the scalar engine with
fused bias + ReLU (towers) / Exp (reg head) and bf16 cast.  Levels 2-4 are
packed into one segmented pass so they share weight streams.  The 5-channel
reg+ctr final conv col-tiles 4 spatial tiles concurrently across PE column
groups.

Self-contained: only library imports (concourse et al.), shapes hardcoded.
"""

import numpy as np
import ml_dtypes

import concourse.bacc as bacc
import concourse.mybir as mybir
import concourse.tile as tile
from concourse.bass_utils import run_bass_kernel_spmd

AF = mybir.ActivationFunctionType
F32 = mybir.dt.float32
BF16 = mybir.dt.bfloat16
BF16_NP = ml_dtypes.bfloat16

LEVELS = [(100, 152), (50, 76), (25, 38), (13, 19), (7, 10)]
B, C, NCLS, STACKED = 8, 256, 80, 4
N_CORES = 8
PSUM_FREE = 512
SEG_GROUPS = [[0], [1], [2, 3, 4]]  # levels packed per processing pass

_CACHE = {}


def _level_tiles(H, W):
    """Row-aligned PSUM tiles: (r0, nr), nr*(W+2) <= PSUM_FREE."""
    Wp = W + 2
    nrmax = min(PSUM_FREE // Wp, H)
    out = []
    r0 = 1
    while r0 <= H:
        nr = min(nrmax, H - r0 + 1)
        out.append((r0, nr))
        r0 += nr
    return out


def _segments(group):
    """[(level, H, W, flat_offset)] with +2 margin cells per segment."""
    segs = []
    off = 0
    for l in group:
        H, W = LEVELS[l]
        segs.append((l, H, W, off))
        off += (H + 2) * (W + 2) + 2
    return segs


def build_nc():
    nc = bacc.Bacc(trn_type="TRN2", num_swdge_queues=4)

    xs = [
        nc.dram_tensor(f"x{i}", [C, h, w], BF16, kind="ExternalInput")
        for i, (h, w) in enumerate(LEVELS)
    ]
    tw = nc.dram_tensor("tw", [2, STACKED, 2, 128, 9 * 256], BF16, kind="ExternalInput")
    fwc = nc.dram_tensor("fw_cls", [2, 128, 9, NCLS], BF16, kind="ExternalInput")
    fwr = nc.dram_tensor("fw_rc", [2, 128, 9, 5], BF16, kind="ExternalInput")
    tb = nc.dram_tensor("tb", [128, 16], F32, kind="ExternalInput")
    fbc = nc.dram_tensor("fb_cls", [128, 3], F32, kind="ExternalInput")
    fbr = nc.dram_tensor("fb_rc", [128, 5], F32, kind="ExternalInput")
    fsr = nc.dram_tensor("fs_rc", [128, 5], F32, kind="ExternalInput")

    ocs = [
        nc.dram_tensor(f"oc{l}", [NCLS, h, w], F32, kind="ExternalOutput")
        for l, (h, w) in enumerate(LEVELS)
    ]
    ors = [
        nc.dram_tensor(f"or{l}", [4, h, w], F32, kind="ExternalOutput")
        for l, (h, w) in enumerate(LEVELS)
    ]
    ots = [
        nc.dram_tensor(f"ot{l}", [1, h, w], F32, kind="ExternalOutput")
        for l, (h, w) in enumerate(LEVELS)
    ]

    SZ0 = max(
        sum((h + 2) * (w + 2) + 2 for _, h, w, _ in _segments(g)) for g in SEG_GROUPS
    )

    with tile.TileContext(nc) as tc:
        with (
            tc.tile_pool(name="wpool", bufs=1) as wpool,
            tc.tile_pool(name="wstream", bufs=5) as wstream,
            tc.tile_pool(name="abuf", bufs=1) as abuf,
            tc.tile_pool(name="psum", bufs=8, space="PSUM") as psum,
            tc.tile_pool(name="stage", bufs=4) as stage,
        ):
            # ---- activation buffers: P/Q ping-pong -----------------------
            bufP = [abuf.tile([128, SZ0], BF16, name=f"P{c}", tag=f"P{c}") for c in range(2)]
            bufQ = [abuf.tile([128, SZ0], BF16, name=f"Q{c}", tag=f"Q{c}") for c in range(2)]

            def memset_pads(buf, group):
                for _, H, W, off in _segments(group):
                    Wp = W + 2
                    S = (H + 2) * Wp
                    for c in range(2):
                        nc.vector.memset(buf[c][:, off : off + Wp + 2], 0.0)
                        nc.vector.memset(
                            buf[c][:, off + 1 + (H + 1) * Wp : off + S + 2], 0.0
                        )
                        colpads = buf[c][:, off + 2 * Wp : off + 2 * Wp + H * Wp]
                        nc.vector.memset(
                            colpads.rearrange("p (r w) -> p r w", w=Wp)[:, :, 0:2],
                            0.0,
                        )

            def load_x(buf, group):
                for l, H, W, off in _segments(group):
                    Wp = W + 2
                    if l == 0:
                        bands = [(0, 8), (8, 33), (33, 58), (58, 79), (79, H)]
                    elif H > 8:
                        hb = (H + 1) // 2
                        bands = [(0, hb), (hb, H)]
                    else:
                        bands = [(0, H)]
                    for r0, r1 in bands:
                        for c in range(2):
                            dst = buf[c][
                                :,
                                off + Wp + 2 + r0 * Wp : off + Wp + 2 + r1 * Wp,
                            ].rearrange("p (r w) -> p r w", w=Wp)[:, :, 0:W]
                            nc.sync.dma_start(
                                dst, xs[l][c * 128 : (c + 1) * 128, r0:r1, :]
                            )

            def load_tw(t, j):
                w_tile = wstream.tile([128, 2, 9, 256], BF16, name="tws", tag="tws")
                for ci in range(2):
                    nc.gpsimd.dma_start(
                        w_tile[:, ci, :, :],
                        tw[t, j, ci, :, :].rearrange("p (k m) -> p k m", k=9),
                    )
                return w_tile

            def conv_tower(src, dst, t, j, group):
                """dst = relu(conv3x3(src) + b) for every segment of group."""
                bcol0 = (t * STACKED + j) * 2
                wt = load_tw(t, j)
                for _, H, W, off in _segments(group):
                    Wp = W + 2
                    for r0, nr in _level_tiles(H, W):
                        N = nr * Wp
                        base = off + 1 + r0 * Wp
                        for co in range(2):
                            ps = psum.tile([128, N], F32, name="ps", tag="ps")
                            kk = 0
                            for ci in range(2):
                                for dy in range(3):
                                    for dx in range(3):
                                        st = base + (dy - 1) * Wp + (dx - 1)
                                        nc.tensor.matmul(
                                            ps[:, 0:N],
                                            wt[:, ci, dy * 3 + dx, co * 128 : (co + 1) * 128],
                                            src[ci][:, st : st + N],
                                            start=(kk == 0),
                                            stop=(kk == 17),
                                        )
                                        kk += 1
                            ps_v = ps[:, 0:N].rearrange("p (r w) -> p r w", w=Wp)[
                                :, :, 1 : 1 + W
                            ]
                            dst_v = dst[co][:, base : base + N].rearrange(
                                "p (r w) -> p r w", w=Wp
                            )[:, :, 1 : 1 + W]
                            nc.scalar.activation(
                                dst_v,
                                ps_v,
                                AF.Relu,
                                bias=tbt[:, bcol0 + co : bcol0 + co + 1],
                            )

            def conv_final_cls(src, group):
                # col-tile: split 80 couts into 32+32+16 chains; pack chains
                # of 4 consecutive spatial tiles across PE column groups so
                # 4 tiles cost 3 stream slots instead of 4.
                jobs = []
                for l, H, W, off in _segments(group):
                    for r0, nr in _level_tiles(H, W):
                        jobs.append((l, H, W, off, r0, nr))
                PARTS = [(0, 32), (32, 32), (64, 16)]
                for q0 in range(0, len(jobs), 4):
                    quad = jobs[q0 : q0 + 4]
                    chains = []  # (slot, grp, job, cout0, m)
                    for ti, job in enumerate(quad):
                        for j, (c0, m) in enumerate(PARTS):
                            c = ti * 3 + j
                            chains.append((c // 4, c % 4, job, c0, m))
                    nslots = (len(chains) + 3) // 4
                    pss = [
                        psum.tile([128, PSUM_FREE], F32, name="ps", tag="ps")
                        for _ in range(nslots)
                    ]
                    kk = 0
                    for ci in range(2):
                        for dy in range(3):
                            for dx in range(3):
                                for slot, grp, (l, H, W, off, r0, nr), c0, m in chains:
                                    Wp = W + 2
                                    N = nr * Wp
                                    st = off + 1 + r0 * Wp + (dy - 1) * Wp + (dx - 1)
                                    nc.tensor.matmul(
                                        pss[slot][32 * grp : 32 * grp + m, 0:N],
                                        fwct[ci][:, dy * 3 + dx, c0 : c0 + m],
                                        src[ci][:, st : st + N],
                                        start=(kk == 0),
                                        stop=(kk == 17),
                                        tile_position=(0, 32 * grp),
                                    )
                                kk += 1
                    sgs = {}
                    for slot, grp, (l, H, W, off, r0, nr), c0, m in chains:
                        Wp = W + 2
                        N = nr * Wp
                        p0 = 32 * grp
                        if slot not in sgs:
                            sgs[slot] = stage.tile(
                                [128, PSUM_FREE], F32, name="sg", tag="sg"
                            )
                        sg = sgs[slot]
                        j = c0 // 32
                        ps_v = pss[slot][p0 : p0 + m, 0:N].rearrange(
                            "p (r w) -> p r w", w=Wp
                        )[:, :, 1 : 1 + W]
                        nc.scalar.activation(
                            sg[p0 : p0 + m, 0 : nr * W].rearrange(
                                "p (r w) -> p r w", w=W
                            ),
                            ps_v,
                            AF.Identity,
                            bias=fbct[p0 : p0 + m, j : j + 1],
                        )
                        oc_flat = ocs[l].ap().rearrange("c h w -> c (h w)")
                        nc.sync.dma_start(
                            oc_flat[c0 : c0 + m, (r0 - 1) * W : (r0 - 1 + nr) * W],
                            sg[p0 : p0 + m, 0 : nr * W],
                        )

            def conv_final_rc(src, group):
                # 4 spatial tiles concurrently via PE column groups 0..3
                jobs = []
                for l, H, W, off in _segments(group):
                    for r0, nr in _level_tiles(H, W):
                        jobs.append((l, H, W, off, r0, nr))
                for q0 in range(0, len(jobs), 4):
                    quad = jobs[q0 : q0 + 4]
                    ps = psum.tile([128, PSUM_FREE], F32, name="ps", tag="ps")
                    kk = 0
                    for ci in range(2):
                        for dy in range(3):
                            for dx in range(3):
                                for g, (l, H, W, off, r0, nr) in enumerate(quad):
                                    Wp = W + 2
                                    N = nr * Wp
                                    st = (
                                        off + 1 + r0 * Wp + (dy - 1) * Wp + (dx - 1)
                                    )
                                    nc.tensor.matmul(
                                        ps[32 * g : 32 * g + 5, 0:N],
                                        fwrt[ci][:, dy * 3 + dx, :],
                                        src[ci][:, st : st + N],
                                        start=(kk == 0),
                                        stop=(kk == 17),
                                        tile_position=(0, 32 * g),
                                    )
                                kk += 1
                    sg = stage.tile([128, PSUM_FREE], F32, name="sg", tag="sg")
                    sg2 = stage.tile([128, PSUM_FREE], F32, name="sg2", tag="sg2")
                    for g, (l, H, W, off, r0, nr) in enumerate(quad):
                        Wp = W + 2
                        N = nr * Wp
                        p0 = 32 * g
                        or_flat = ors[l].ap().rearrange("c h w -> c (h w)")
                        ot_flat = ots[l].ap().rearrange("c h w -> c (h w)")
                        ps_v = ps[p0 : p0 + 5, 0:N].rearrange(
                            "p (r w) -> p r w", w=Wp
                        )[:, :, 1 : 1 + W]
                        # y = scale*conv + bias (reg rows: coef*conv + coef*b;
                        # ctr row: conv + b)
                        nc.scalar.activation(
                            sg[p0 : p0 + 5, 0 : nr * W].rearrange(
                                "p (r w) -> p r w", w=W
                            ),
                            ps_v,
                            AF.Identity,
                            bias=fbrt[p0 : p0 + 5, l : l + 1],
                            scale=fsrt[p0 : p0 + 5, l : l + 1],
                        )
                        nc.scalar.activation(
                            sg2[p0 : p0 + 4, 0 : nr * W],
                            sg[p0 : p0 + 4, 0 : nr * W],
                            AF.Exp,
                        )
                        nc.sync.dma_start(
                            or_flat[:, (r0 - 1) * W : (r0 - 1 + nr) * W],
                            sg2[p0 : p0 + 4, 0 : nr * W],
                        )
                        nc.sync.dma_start(
                            ot_flat[:, (r0 - 1) * W : (r0 - 1 + nr) * W],
                            sg[p0 + 4 : p0 + 5, 0 : nr * W],
                        )

            # ---- first input load (head of the SP DMA ring) --------------
            memset_pads(bufP, SEG_GROUPS[0])
            load_x(bufP, SEG_GROUPS[0])
            memset_pads(bufQ, SEG_GROUPS[0])

            # ---- resident final weights / biases (SWDGE; off the SP ring)
            fwct = []
            fwrt = []
            for ci in range(2):
                w_tile = wpool.tile([128, 9, NCLS], BF16, name=f"fwc{ci}", tag=f"fwc{ci}")
                nc.gpsimd.dma_start(w_tile[:], fwc[ci, :, :, :])
                fwct.append(w_tile)
                w_tile = wpool.tile([128, 9, 5], BF16, name=f"fwr{ci}", tag=f"fwr{ci}")
                nc.gpsimd.dma_start(w_tile[:], fwr[ci, :, :, :])
                fwrt.append(w_tile)
            tbt = wpool.tile([128, 16], F32, name="tb", tag="tb")
            nc.gpsimd.dma_start(tbt[:], tb[:, :])
            fbct = wpool.tile([128, 3], F32, name="fbc", tag="fbc")
            nc.gpsimd.dma_start(fbct[:], fbc[:, :])
            fbrt = wpool.tile([128, 5], F32, name="fbr", tag="fbr")
            nc.gpsimd.dma_start(fbrt[:], fbr[:, :])
            fsrt = wpool.tile([128, 5], F32, name="fsr", tag="fsr")
            nc.gpsimd.dma_start(fsrt[:], fsr[:, :])

            # ---- main schedule: P/Q ping-pong, x re-loaded for reg tower -
            for gi, group in enumerate(SEG_GROUPS):
                conv_tower(bufP, bufQ, 0, 0, group)  # c1 cls (P holds x)
                conv_tower(bufQ, bufP, 0, 1, group)  # c2 cls
                conv_tower(bufP, bufQ, 0, 2, group)  # c3 cls
                conv_tower(bufQ, bufP, 0, 3, group)  # c4 cls
                conv_final_cls(bufP, group)
                load_x(bufQ, group)                  # re-load x (Q free after c4 cls)
                conv_tower(bufQ, bufP, 1, 0, group)  # c1 reg (P free after final cls)
                conv_tower(bufP, bufQ, 1, 1, group)  # c2 reg
                conv_tower(bufQ, bufP, 1, 2, group)  # c3 reg
                conv_tower(bufP, bufQ, 1, 3, group)  # c4 reg
                if gi + 1 < len(SEG_GROUPS):
                    # P free after c4 reg read it: prep next group's input
                    memset_pads(bufP, SEG_GROUPS[gi + 1])
                    load_x(bufP, SEG_GROUPS[gi + 1])
                conv_final_rc(bufQ, group)
                if gi + 1 < len(SEG_GROUPS):
                    memset_pads(bufQ, SEG_GROUPS[gi + 1])

    nc.compile()
    return nc


def _prep_weights(inputs):
    """Host-side: transpose conv weights into lhsT layouts, cast to bf16."""
    cls_w = np.asarray(inputs["cls_w"], np.float32)
    reg_w = np.asarray(inputs["reg_w"], np.float32)
    cls_b = np.asarray(inputs["cls_b"], np.float32)
    reg_b = np.asarray(inputs["reg_b"], np.float32)
    fcls_w = np.asarray(inputs["fcls_w"], np.float32)
    fcls_b = np.asarray(inputs["fcls_b"], np.float32)
    freg_w = np.asarray(inputs["freg_w"], np.float32)
    freg_b = np.asarray(inputs["freg_b"], np.float32)
    fctr_w = np.asarray(inputs["fctr_w"], np.float32)
    fctr_b = np.asarray(inputs["fctr_b"], np.float32)
    reg_coef = np.asarray(inputs["reg_coef"], np.float32)

    tw = np.empty((2, STACKED, 2, 128, 9 * 256), BF16_NP)
    for t, warr in enumerate((cls_w, reg_w)):
        for j in range(STACKED):
            # [O, I, 3, 3] -> [k, I, O]
            A = warr[j].transpose(2, 3, 1, 0).reshape(9, C, C)
            for ci in range(2):
                tw[t, j, ci] = (
                    A[:, ci * 128 : (ci + 1) * 128, :]
                    .transpose(1, 0, 2)
                    .reshape(128, 9 * 256)
                    .astype(BF16_NP)
                )

    def final_lhsT(w):  # [O, C, 3, 3] -> [2, 128, 9, O]
        O = w.shape[0]
        A = w.transpose(2, 3, 1, 0).reshape(9, C, O)
        out = np.empty((2, 128, 9, O), BF16_NP)
        for ci in range(2):
            out[ci] = A[:, ci * 128 : (ci + 1) * 128, :].transpose(1, 0, 2)
        return out

    fw_cls = final_lhsT(fcls_w)
    fw_rc = final_lhsT(np.concatenate([freg_w, fctr_w], axis=0))

    tb = np.zeros((128, 16), np.float32)
    for t, barr in enumerate((cls_b, reg_b)):
        for j in range(STACKED):
            for co in range(2):
                tb[:, (t * STACKED + j) * 2 + co] = barr[j, co * 128 : (co + 1) * 128]
    fb_cls = np.zeros((128, 3), np.float32)
    for g in range(4):
        fb_cls[32 * g : 32 * g + 32, 0] = fcls_b[0:32]
        fb_cls[32 * g : 32 * g + 32, 1] = fcls_b[32:64]
        fb_cls[32 * g : 32 * g + 16, 2] = fcls_b[64:80]
    fb_rc = np.zeros((128, 5), np.float32)
    fs_rc = np.ones((128, 5), np.float32)
    for l in range(len(LEVELS)):
        for g in range(4):
            fb_rc[32 * g : 32 * g + 4, l] = reg_coef[l] * freg_b
            fb_rc[32 * g + 4, l] = fctr_b[0]
            fs_rc[32 * g : 32 * g + 4, l] = reg_coef[l]
    return {
        "tw": tw,
        "fw_cls": fw_cls,
        "fw_rc": fw_rc,
        "tb": tb,
        "fb_cls": fb_cls,
        "fb_rc": fb_rc,
        "fs_rc": fs_rc,
    }


def kernel(**inputs):
    if "nc" not in _CACHE:
        _CACHE["nc"] = build_nc()
    nc = _CACHE["nc"]

    shared = _prep_weights(inputs)
    xs = [np.asarray(inputs[f"x{i}"], np.float32) for i in range(len(LEVELS))]
    xs_b = [x.astype(BF16_NP) for x in xs]

    in_maps = []
    for c in range(N_CORES):
        m = dict(shared)
        for i in range(len(LEVELS)):
            m[f"x{i}"] = np.ascontiguousarray(xs_b[i][c])
        in_maps.append(m)

    res = run_bass_kernel_spmd(nc, in_maps, core_ids=list(range(N_CORES)))

    cls_outs, reg_outs, ctr_outs = [], [], []
    for l in range(len(LEVELS)):
        cls_outs.append(np.stack([res.results[c][f"oc{l}"] for c in range(N_CORES)]))
        reg_outs.append(np.stack([res.results[c][f"or{l}"] for c in range(N_CORES)]))
        ctr_outs.append(np.stack([res.results[c][f"ot{l}"] for c in range(N_CORES)]))
    return tuple(cls_outs + reg_outs + ctr_outs)


# revision 15
# speedup vs baseline: 1.0271x; 1.0078x over previous
"""FCOS head (nn_FCOSHead_60249801228382) Trainium2 Bass kernel.

Data-parallel over batch: 8 images -> 8 NeuronCores, conv weights replicated.
Per core, FPN levels are processed in a padded flat activation layout in SBUF;
every 3x3 conv is computed as 18 accumulating matmuls (9 spatial taps x 2
input-channel chunks of 128) into PSUM, drained by the scalar engine with
fused bias + ReLU (towers) / Exp (reg head) and bf16 cast.  Levels 2-4 are
packed into one segmented pass so they share weight streams.  The 5-channel
reg+ctr final conv col-tiles 4 spatial tiles concurrently across PE column
groups.

Self-contained: only library imports (concourse et al.), shapes hardcoded.
"""

import numpy as np
import ml_dtypes

import concourse.bacc as bacc
import concourse.mybir as mybir
import concourse.tile as tile
from concourse.bass_utils import run_bass_kernel_spmd

AF = mybir.ActivationFunctionType
F32 = mybir.dt.float32
BF16 = mybir.dt.bfloat16
BF16_NP = ml_dtypes.bfloat16

LEVELS = [(100, 152), (50, 76), (25, 38), (13, 19), (7, 10)]
B, C, NCLS, STACKED = 8, 256, 80, 4
N_CORES = 8
PSUM_FREE = 512
SEG_GROUPS = [[0], [1], [2, 3, 4]]  # levels packed per processing pass

_CACHE = {}


def _level_tiles(H, W):
    """Row-aligned PSUM tiles: (r0, nr), nr*(W+1) <= PSUM_FREE."""
    Wp = W + 1
    nrmax = min(PSUM_FREE // Wp, H)
    out = []
    r0 = 1
    while r0 <= H:
        nr = min(nrmax, H - r0 + 1)
        out.append((r0, nr))
        r0 += nr
    return out


def _segments(group):
    """[(level, H, W, flat_offset)] with +2 margin cells per segment."""
    segs = []
    off = 0
    for l in group:
        H, W = LEVELS[l]
        segs.append((l, H, W, off))
        off += (H + 2) * (W + 1) + 2
    return segs


def build_nc():
    nc = bacc.Bacc(trn_type="TRN2", num_swdge_queues=4)

    xs = [
        nc.dram_tensor(f"x{i}", [C, h, w], BF16, kind="ExternalInput")
        for i, (h, w) in enumerate(LEVELS)
    ]
    tw = nc.dram_tensor("tw", [2, STACKED, 2, 128, 9 * 256], BF16, kind="ExternalInput")
    fwc = nc.dram_tensor("fw_cls", [2, 128, 9, NCLS], BF16, kind="ExternalInput")
    fwr = nc.dram_tensor("fw_rc", [2, 128, 9, 5], BF16, kind="ExternalInput")
    tb = nc.dram_tensor("tb", [128, 16], F32, kind="ExternalInput")
    fbc = nc.dram_tensor("fb_cls", [128, 3], F32, kind="ExternalInput")
    fbr = nc.dram_tensor("fb_rc", [128, 5], F32, kind="ExternalInput")
    fsr = nc.dram_tensor("fs_rc", [128, 5], F32, kind="ExternalInput")

    ocs = [
        nc.dram_tensor(f"oc{l}", [NCLS, h, w], F32, kind="ExternalOutput")
        for l, (h, w) in enumerate(LEVELS)
    ]
    ors = [
        nc.dram_tensor(f"or{l}", [4, h, w], F32, kind="ExternalOutput")
        for l, (h, w) in enumerate(LEVELS)
    ]
    ots = [
        nc.dram_tensor(f"ot{l}", [1, h, w], F32, kind="ExternalOutput")
        for l, (h, w) in enumerate(LEVELS)
    ]

    SZ0 = max(
        sum((h + 2) * (w + 1) + 2 for _, h, w, _ in _segments(g)) for g in SEG_GROUPS
    )

    with tile.TileContext(nc) as tc:
        with (
            tc.tile_pool(name="wpool", bufs=1) as wpool,
            tc.tile_pool(name="wstream", bufs=5) as wstream,
            tc.tile_pool(name="abuf", bufs=1) as abuf,
            tc.tile_pool(name="psum", bufs=8, space="PSUM") as psum,
            tc.tile_pool(name="stage", bufs=4) as stage,
        ):
            # ---- activation buffers: P/Q ping-pong -----------------------
            bufP = [abuf.tile([128, SZ0], BF16, name=f"P{c}", tag=f"P{c}") for c in range(2)]
            bufQ = [abuf.tile([128, SZ0], BF16, name=f"Q{c}", tag=f"Q{c}") for c in range(2)]

            def memset_pads(buf, group):
                for _, H, W, off in _segments(group):
                    Wp = W + 1
                    S = (H + 2) * Wp
                    for c in range(2):
                        nc.vector.memset(buf[c][:, off : off + 1 + Wp], 0.0)
                        nc.vector.memset(
                            buf[c][:, off + 1 + (H + 1) * Wp : off + S + 2], 0.0
                        )
                        colpads = buf[c][
                            :, off + 1 + Wp + W : off + 1 + Wp + W + H * Wp
                        ]
                        nc.vector.memset(
                            colpads.rearrange("p (r w) -> p r w", w=Wp)[:, :, 0:1],
                            0.0,
                        )

            def load_x(buf, group):
                for l, H, W, off in _segments(group):
                    Wp = W + 1
                    if l == 0:
                        bands = [(0, 8), (8, 33), (33, 58), (58, 79), (79, H)]
                    elif H > 8:
                        hb = (H + 1) // 2
                        bands = [(0, hb), (hb, H)]
                    else:
                        bands = [(0, H)]
                    for r0, r1 in bands:
                        for c in range(2):
                            dst = buf[c][
                                :,
                                off + 1 + (1 + r0) * Wp : off + 1 + (1 + r1) * Wp,
                            ].rearrange("p (r w) -> p r w", w=Wp)[:, :, 0:W]
                            nc.sync.dma_start(
                                dst, xs[l][c * 128 : (c + 1) * 128, r0:r1, :]
                            )

            def load_tw(t, j):
                w_tile = wstream.tile([128, 2, 9, 256], BF16, name="tws", tag="tws")
                for ci in range(2):
                    nc.gpsimd.dma_start(
                        w_tile[:, ci, :, :],
                        tw[t, j, ci, :, :].rearrange("p (k m) -> p k m", k=9),
                    )
                return w_tile

            def conv_tower(src, dst, t, j, group):
                """dst = relu(conv3x3(src) + b) for every segment of group."""
                bcol0 = (t * STACKED + j) * 2
                wt = load_tw(t, j)
                for _, H, W, off in _segments(group):
                    Wp = W + 1
                    for r0, nr in _level_tiles(H, W):
                        N = nr * Wp
                        base = off + 1 + r0 * Wp
                        for co in range(2):
                            ps = psum.tile([128, N], F32, name="ps", tag="ps")
                            kk = 0
                            for ci in range(2):
                                for dy in range(3):
                                    for dx in range(3):
                                        st = base + (dy - 1) * Wp + (dx - 1)
                                        nc.tensor.matmul(
                                            ps[:, 0:N],
                                            wt[:, ci, dy * 3 + dx, co * 128 : (co + 1) * 128],
                                            src[ci][:, st : st + N],
                                            start=(kk == 0),
                                            stop=(kk == 17),
                                        )
                                        kk += 1
                            ps_v = ps[:, 0:N].rearrange("p (r w) -> p r w", w=Wp)[
                                :, :, 0:W
                            ]
                            dst_v = dst[co][:, base : base + N].rearrange(
                                "p (r w) -> p r w", w=Wp
                            )[:, :, 0:W]
                            nc.scalar.activation(
                                dst_v,
                                ps_v,
                                AF.Relu,
                                bias=tbt[:, bcol0 + co : bcol0 + co + 1],
                            )

            def conv_final_cls(src, group):
                # col-tile: split 80 couts into 32+32+16 chains; pack chains
                # of 4 consecutive spatial tiles across PE column groups so
                # 4 tiles cost 3 stream slots instead of 4.
                jobs = []
                for l, H, W, off in _segments(group):
                    for r0, nr in _level_tiles(H, W):
                        jobs.append((l, H, W, off, r0, nr))
                PARTS = [(0, 32), (32, 32), (64, 16)]
                for q0 in range(0, len(jobs), 4):
                    quad = jobs[q0 : q0 + 4]
                    chains = []  # (slot, grp, job, cout0, m)
                    for ti, job in enumerate(quad):
                        for j, (c0, m) in enumerate(PARTS):
                            c = ti * 3 + j
                            chains.append((c // 4, c % 4, job, c0, m))
                    nslots = (len(chains) + 3) // 4
                    pss = [
                        psum.tile([128, PSUM_FREE], F32, name="ps", tag="ps")
                        for _ in range(nslots)
                    ]
                    kk = 0
                    for ci in range(2):
                        for dy in range(3):
                            for dx in range(3):
                                for slot, grp, (l, H, W, off, r0, nr), c0, m in chains:
                                    Wp = W + 1
                                    N = nr * Wp
                                    st = off + 1 + r0 * Wp + (dy - 1) * Wp + (dx - 1)
                                    nc.tensor.matmul(
                                        pss[slot][32 * grp : 32 * grp + m, 0:N],
                                        fwct[ci][:, dy * 3 + dx, c0 : c0 + m],
                                        src[ci][:, st : st + N],
                                        start=(kk == 0),
                                        stop=(kk == 17),
                                        tile_position=(0, 32 * grp),
                                    )
                                kk += 1
                    sgs = {}
                    for slot, grp, (l, H, W, off, r0, nr), c0, m in chains:
                        Wp = W + 1
                        N = nr * Wp
                        p0 = 32 * grp
                        if slot not in sgs:
                            sgs[slot] = stage.tile(
                                [128, PSUM_FREE], F32, name="sg", tag="sg"
                            )
                        sg = sgs[slot]
                        j = c0 // 32
                        ps_v = pss[slot][p0 : p0 + m, 0:N].rearrange(
                            "p (r w) -> p r w", w=Wp
                        )[:, :, 0:W]
                        nc.scalar.activation(
                            sg[p0 : p0 + m, 0 : nr * W].rearrange(
                                "p (r w) -> p r w", w=W
                            ),
                            ps_v,
                            AF.Identity,
                            bias=fbct[p0 : p0 + m, j : j + 1],
                        )
                        oc_flat = ocs[l].ap().rearrange("c h w -> c (h w)")
                        nc.sync.dma_start(
                            oc_flat[c0 : c0 + m, (r0 - 1) * W : (r0 - 1 + nr) * W],
                            sg[p0 : p0 + m, 0 : nr * W],
                        )

            def conv_final_rc(src, group):
                # 4 spatial tiles concurrently via PE column groups 0..3
                jobs = []
                for l, H, W, off in _segments(group):
                    for r0, nr in _level_tiles(H, W):
                        jobs.append((l, H, W, off, r0, nr))
                for q0 in range(0, len(jobs), 4):
                    quad = jobs[q0 : q0 + 4]
                    ps = psum.tile([128, PSUM_FREE], F32, name="ps", tag="ps")
                    kk = 0
                    for ci in range(2):
                        for dy in range(3):
                            for dx in range(3):
                                for g, (l, H, W, off, r0, nr) in enumerate(quad):
                                    Wp = W + 1
                                    N = nr * Wp
                                    st = (
                                        off + 1 + r0 * Wp + (dy - 1) * Wp + (dx - 1)
                                    )
                                    nc.tensor.matmul(
                                        ps[32 * g : 32 * g + 5, 0:N],
                                        fwrt[ci][:, dy * 3 + dx, :],
                                        src[ci][:, st : st + N],
                                        start=(kk == 0),
                                        stop=(kk == 17),
                                        tile_position=(0, 32 * g),
                                    )
                                kk += 1
                    sg = stage.tile([128, PSUM_FREE], F32, name="sg", tag="sg")
                    sg2 = stage.tile([128, PSUM_FREE], F32, name="sg2", tag="sg2")
                    for g, (l, H, W, off, r0, nr) in enumerate(quad):
                        Wp = W + 1
                        N = nr * Wp
                        p0 = 32 * g
                        or_flat = ors[l].ap().rearrange("c h w -> c (h w)")
                        ot_flat = ots[l].ap().rearrange("c h w -> c (h w)")
                        ps_v = ps[p0 : p0 + 5, 0:N].rearrange(
                            "p (r w) -> p r w", w=Wp
                        )[:, :, 0:W]
                        # y = scale*conv + bias (reg rows: coef*conv + coef*b;
                        # ctr row: conv + b)
                        nc.scalar.activation(
                            sg[p0 : p0 + 5, 0 : nr * W].rearrange(
                                "p (r w) -> p r w", w=W
                            ),
                            ps_v,
                            AF.Identity,
                            bias=fbrt[p0 : p0 + 5, l : l + 1],
                            scale=fsrt[p0 : p0 + 5, l : l + 1],
                        )
                        nc.scalar.activation(
                            sg2[p0 : p0 + 4, 0 : nr * W],
                            sg[p0 : p0 + 4, 0 : nr * W],
                            AF.Exp,
                        )
                        nc.sync.dma_start(
                            or_flat[:, (r0 - 1) * W : (r0 - 1 + nr) * W],
                            sg2[p0 : p0 + 4, 0 : nr * W],
                        )
                        nc.sync.dma_start(
                            ot_flat[:, (r0 - 1) * W : (r0 - 1 + nr) * W],
                            sg[p0 + 4 : p0 + 5, 0 : nr * W],
                        )

            # ---- first input load (head of the SP DMA ring) --------------
            memset_pads(bufP, SEG_GROUPS[0])
            load_x(bufP, SEG_GROUPS[0])
            memset_pads(bufQ, SEG_GROUPS[0])

            # ---- resident final weights / biases (SWDGE; off the SP ring)
            fwct = []
            fwrt = []
            for ci in range(2):
                w_tile = wpool.tile([128, 9, NCLS], BF16, name=f"fwc{ci}", tag=f"fwc{ci}")
                nc.gpsimd.dma_start(w_tile[:], fwc[ci, :, :, :])
                fwct.append(w_tile)
                w_tile = wpool.tile([128, 9, 5], BF16, name=f"fwr{ci}", tag=f"fwr{ci}")
                nc.gpsimd.dma_start(w_tile[:], fwr[ci, :, :, :])
                fwrt.append(w_tile)
            tbt = wpool.tile([128, 16], F32, name="tb", tag="tb")
            nc.gpsimd.dma_start(tbt[:], tb[:, :])
            fbct = wpool.tile([128, 3], F32, name="fbc", tag="fbc")
            nc.gpsimd.dma_start(fbct[:], fbc[:, :])
            fbrt = wpool.tile([128, 5], F32, name="fbr", tag="fbr")
            nc.gpsimd.dma_start(fbrt[:], fbr[:, :])
            fsrt = wpool.tile([128, 5], F32, name="fsr", tag="fsr")
            nc.gpsimd.dma_start(fsrt[:], fsr[:, :])

            # ---- main schedule: P/Q ping-pong, x re-loaded for reg tower -
            for gi, group in enumerate(SEG_GROUPS):
                conv_tower(bufP, bufQ, 0, 0, group)  # c1 cls (P holds x)
                conv_tower(bufQ, bufP, 0, 1, group)  # c2 cls
                conv_tower(bufP, bufQ, 0, 2, group)  # c3 cls
                conv_tower(bufQ, bufP, 0, 3, group)  # c4 cls
                conv_final_cls(bufP, group)
                load_x(bufQ, group)                  # re-load x (Q free after c4 cls)
                conv_tower(bufQ, bufP, 1, 0, group)  # c1 reg (P free after final cls)
                conv_tower(bufP, bufQ, 1, 1, group)  # c2 reg
                conv_tower(bufQ, bufP, 1, 2, group)  # c3 reg
                conv_tower(bufP, bufQ, 1, 3, group)  # c4 reg
                if gi + 1 < len(SEG_GROUPS):
                    # P free after c4 reg read it: prep next group's input
                    memset_pads(bufP, SEG_GROUPS[gi + 1])
                    load_x(bufP, SEG_GROUPS[gi + 1])
                conv_final_rc(bufQ, group)
                if gi + 1 < len(SEG_GROUPS):
                    memset_pads(bufQ, SEG_GROUPS[gi + 1])

    nc.compile()
    return nc


def _prep_weights(inputs):
    """Host-side: transpose conv weights into lhsT layouts, cast to bf16."""
    cls_w = np.asarray(inputs["cls_w"], np.float32)
    reg_w = np.asarray(inputs["reg_w"], np.float32)
    cls_b = np.asarray(inputs["cls_b"], np.float32)
    reg_b = np.asarray(inputs["reg_b"], np.float32)
    fcls_w = np.asarray(inputs["fcls_w"], np.float32)
    fcls_b = np.asarray(inputs["fcls_b"], np.float32)
    freg_w = np.asarray(inputs["freg_w"], np.float32)
    freg_b = np.asarray(inputs["freg_b"], np.float32)
    fctr_w = np.asarray(inputs["fctr_w"], np.float32)
    fctr_b = np.asarray(inputs["fctr_b"], np.float32)
    reg_coef = np.asarray(inputs["reg_coef"], np.float32)

    tw = np.empty((2, STACKED, 2, 128, 9 * 256), BF16_NP)
    for t, warr in enumerate((cls_w, reg_w)):
        for j in range(STACKED):
            # [O, I, 3, 3] -> [k, I, O]
            A = warr[j].transpose(2, 3, 1, 0).reshape(9, C, C)
            for ci in range(2):
                tw[t, j, ci] = (
                    A[:, ci * 128 : (ci + 1) * 128, :]
                    .transpose(1, 0, 2)
                    .reshape(128, 9 * 256)
                    .astype(BF16_NP)
                )

    def final_lhsT(w):  # [O, C, 3, 3] -> [2, 128, 9, O]
        O = w.shape[0]
        A = w.transpose(2, 3, 1, 0).reshape(9, C, O)
        out = np.empty((2, 128, 9, O), BF16_NP)
        for ci in range(2):
            out[ci] = A[:, ci * 128 : (ci + 1) * 128, :].transpose(1, 0, 2)
        return out

    fw_cls = final_lhsT(fcls_w)
    fw_rc = final_lhsT(np.concatenate([freg_w, fctr_w], axis=0))

    tb = np.zeros((128, 16), np.float32)
    for t, barr in enumerate((cls_b, reg_b)):
        for j in range(STACKED):
            for co in range(2):
                tb[:, (t * STACKED + j) * 2 + co] = barr[j, co * 128 : (co + 1) * 128]
    fb_cls = np.zeros((128, 3), np.float32)
    for g in range(4):
        fb_cls[32 * g : 32 * g + 32, 0] = fcls_b[0:32]
        fb_cls[32 * g : 32 * g + 32, 1] = fcls_b[32:64]
        fb_cls[32 * g : 32 * g + 16, 2] = fcls_b[64:80]
    fb_rc = np.zeros((128, 5), np.float32)
    fs_rc = np.ones((128, 5), np.float32)
    for l in range(len(LEVELS)):
        for g in range(4):
            fb_rc[32 * g : 32 * g + 4, l] = reg_coef[l] * freg_b
            fb_rc[32 * g + 4, l] = fctr_b[0]
            fs_rc[32 * g : 32 * g + 4, l] = reg_coef[l]
    return {
        "tw": tw,
        "fw_cls": fw_cls,
        "fw_rc": fw_rc,
        "tb": tb,
        "fb_cls": fb_cls,
        "fb_rc": fb_rc,
        "fs_rc": fs_rc,
    }


def kernel(**inputs):
    if "nc" not in _CACHE:
        _CACHE["nc"] = build_nc()
    nc = _CACHE["nc"]

    shared = _prep_weights(inputs)
    xs = [np.asarray(inputs[f"x{i}"], np.float32) for i in range(len(LEVELS))]
    xs_b = [x.astype(BF16_NP) for x in xs]

    in_maps = []
    for c in range(N_CORES):
        m = dict(shared)
        for i in range(len(LEVELS)):
            m[f"x{i}"] = np.ascontiguousarray(xs_b[i][c])
        in_maps.append(m)

    res = run_bass_kernel_spmd(nc, in_maps, core_ids=list(range(N_CORES)))

    cls_outs, reg_outs, ctr_outs = [], [], []
    for l in range(len(LEVELS)):
        cls_outs.append(np.stack([res.results[c][f"oc{l}"] for c in range(N_CORES)]))
        reg_outs.append(np.stack([res.results[c][f"or{l}"] for c in range(N_CORES)]))
        ctr_outs.append(np.stack([res.results[c][f"ot{l}"] for c in range(N_CORES)]))
    return tuple(cls_outs + reg_outs + ctr_outs)


# revision 16
# speedup vs baseline: 1.0278x; 1.0006x over previous
"""FCOS head (nn_FCOSHead_60249801228382) Trainium2 Bass kernel.

Data-parallel over batch: 8 images -> 8 NeuronCores, conv weights replicated.
Per core, FPN levels are processed in a padded flat activation layout in SBUF;
every 3x3 conv is computed as 18 accumulating matmuls (9 spatial taps x 2
input-channel chunks of 128) into PSUM, drained by the scalar engine with
fused bias + ReLU (towers) / Exp (reg head) and bf16 cast.  Levels 2-4 are
packed into one segmented pass so they share weight streams.  The 5-channel
reg+ctr final conv col-tiles 4 spatial tiles concurrently across PE column
groups.

Self-contained: only library imports (concourse et al.), shapes hardcoded.
"""

import numpy as np
import ml_dtypes

import concourse.bacc as bacc
import concourse.mybir as mybir
import concourse.tile as tile
from concourse.bass_utils import run_bass_kernel_spmd

AF = mybir.ActivationFunctionType
F32 = mybir.dt.float32
BF16 = mybir.dt.bfloat16
BF16_NP = ml_dtypes.bfloat16

LEVELS = [(100, 152), (50, 76), (25, 38), (13, 19), (7, 10)]
B, C, NCLS, STACKED = 8, 256, 80, 4
N_CORES = 8
PSUM_FREE = 512
SEG_GROUPS = [[0], [1], [2, 3, 4]]  # levels packed per processing pass

_CACHE = {}


def _level_tiles(H, W):
    """Row-aligned PSUM tiles: (r0, nr), nr*(W+1) <= PSUM_FREE.  Rows are
    balanced across tiles (within 1 of each other) so no tile is so short
    that its matmuls go LDWEIGHTS-bound."""
    Wp = W + 1
    nrmax = min(PSUM_FREE // Wp, H)
    n_tiles = -(-H // nrmax)
    base, rem = divmod(H, n_tiles)
    out = []
    r0 = 1
    for i in range(n_tiles):
        nr = base + (1 if i < rem else 0)
        out.append((r0, nr))
        r0 += nr
    return out


def _segments(group):
    """[(level, H, W, flat_offset)] with +2 margin cells per segment."""
    segs = []
    off = 0
    for l in group:
        H, W = LEVELS[l]
        segs.append((l, H, W, off))
        off += (H + 2) * (W + 1) + 2
    return segs


def build_nc():
    nc = bacc.Bacc(trn_type="TRN2", num_swdge_queues=4)

    xs = [
        nc.dram_tensor(f"x{i}", [C, h, w], BF16, kind="ExternalInput")
        for i, (h, w) in enumerate(LEVELS)
    ]
    tw = nc.dram_tensor("tw", [2, STACKED, 2, 128, 9 * 256], BF16, kind="ExternalInput")
    fwc = nc.dram_tensor("fw_cls", [2, 128, 9, NCLS], BF16, kind="ExternalInput")
    fwr = nc.dram_tensor("fw_rc", [2, 128, 9, 5], BF16, kind="ExternalInput")
    tb = nc.dram_tensor("tb", [128, 16], F32, kind="ExternalInput")
    fbc = nc.dram_tensor("fb_cls", [128, 3], F32, kind="ExternalInput")
    fbr = nc.dram_tensor("fb_rc", [128, 5], F32, kind="ExternalInput")
    fsr = nc.dram_tensor("fs_rc", [128, 5], F32, kind="ExternalInput")

    ocs = [
        nc.dram_tensor(f"oc{l}", [NCLS, h, w], F32, kind="ExternalOutput")
        for l, (h, w) in enumerate(LEVELS)
    ]
    ors = [
        nc.dram_tensor(f"or{l}", [4, h, w], F32, kind="ExternalOutput")
        for l, (h, w) in enumerate(LEVELS)
    ]
    ots = [
        nc.dram_tensor(f"ot{l}", [1, h, w], F32, kind="ExternalOutput")
        for l, (h, w) in enumerate(LEVELS)
    ]

    SZ0 = max(
        sum((h + 2) * (w + 1) + 2 for _, h, w, _ in _segments(g)) for g in SEG_GROUPS
    )

    with tile.TileContext(nc) as tc:
        with (
            tc.tile_pool(name="wpool", bufs=1) as wpool,
            tc.tile_pool(name="wstream", bufs=5) as wstream,
            tc.tile_pool(name="abuf", bufs=1) as abuf,
            tc.tile_pool(name="psum", bufs=8, space="PSUM") as psum,
            tc.tile_pool(name="stage", bufs=4) as stage,
        ):
            # ---- activation buffers: P/Q ping-pong -----------------------
            bufP = [abuf.tile([128, SZ0], BF16, name=f"P{c}", tag=f"P{c}") for c in range(2)]
            bufQ = [abuf.tile([128, SZ0], BF16, name=f"Q{c}", tag=f"Q{c}") for c in range(2)]

            def memset_pads(buf, group):
                for _, H, W, off in _segments(group):
                    Wp = W + 1
                    S = (H + 2) * Wp
                    for c in range(2):
                        nc.vector.memset(buf[c][:, off : off + 1 + Wp], 0.0)
                        nc.vector.memset(
                            buf[c][:, off + 1 + (H + 1) * Wp : off + S + 2], 0.0
                        )
                        colpads = buf[c][
                            :, off + 1 + Wp + W : off + 1 + Wp + W + H * Wp
                        ]
                        nc.vector.memset(
                            colpads.rearrange("p (r w) -> p r w", w=Wp)[:, :, 0:1],
                            0.0,
                        )

            def load_x(buf, group):
                for l, H, W, off in _segments(group):
                    Wp = W + 1
                    if l == 0:
                        bands = [(0, 8), (8, 33), (33, 58), (58, 79), (79, H)]
                    elif H > 8:
                        hb = (H + 1) // 2
                        bands = [(0, hb), (hb, H)]
                    else:
                        bands = [(0, H)]
                    for r0, r1 in bands:
                        for c in range(2):
                            dst = buf[c][
                                :,
                                off + 1 + (1 + r0) * Wp : off + 1 + (1 + r1) * Wp,
                            ].rearrange("p (r w) -> p r w", w=Wp)[:, :, 0:W]
                            nc.sync.dma_start(
                                dst, xs[l][c * 128 : (c + 1) * 128, r0:r1, :]
                            )

            def load_tw(t, j):
                w_tile = wstream.tile([128, 2, 9, 256], BF16, name="tws", tag="tws")
                for ci in range(2):
                    nc.gpsimd.dma_start(
                        w_tile[:, ci, :, :],
                        tw[t, j, ci, :, :].rearrange("p (k m) -> p k m", k=9),
                    )
                return w_tile

            def conv_tower(src, dst, t, j, group):
                """dst = relu(conv3x3(src) + b) for every segment of group."""
                bcol0 = (t * STACKED + j) * 2
                wt = load_tw(t, j)
                for _, H, W, off in _segments(group):
                    Wp = W + 1
                    for r0, nr in _level_tiles(H, W):
                        N = nr * Wp
                        base = off + 1 + r0 * Wp
                        for co in range(2):
                            ps = psum.tile([128, N], F32, name="ps", tag="ps")
                            kk = 0
                            for ci in range(2):
                                for dy in range(3):
                                    for dx in range(3):
                                        st = base + (dy - 1) * Wp + (dx - 1)
                                        nc.tensor.matmul(
                                            ps[:, 0:N],
                                            wt[:, ci, dy * 3 + dx, co * 128 : (co + 1) * 128],
                                            src[ci][:, st : st + N],
                                            start=(kk == 0),
                                            stop=(kk == 17),
                                        )
                                        kk += 1
                            ps_v = ps[:, 0:N].rearrange("p (r w) -> p r w", w=Wp)[
                                :, :, 0:W
                            ]
                            dst_v = dst[co][:, base : base + N].rearrange(
                                "p (r w) -> p r w", w=Wp
                            )[:, :, 0:W]
                            nc.scalar.activation(
                                dst_v,
                                ps_v,
                                AF.Relu,
                                bias=tbt[:, bcol0 + co : bcol0 + co + 1],
                            )

            def conv_final_cls(src, group):
                # col-tile: split 80 couts into 32+32+16 chains; pack chains
                # of 4 consecutive spatial tiles across PE column groups so
                # 4 tiles cost 3 stream slots instead of 4.
                jobs = []
                for l, H, W, off in _segments(group):
                    for r0, nr in _level_tiles(H, W):
                        jobs.append((l, H, W, off, r0, nr))
                PARTS = [(0, 32), (32, 32), (64, 16)]
                for q0 in range(0, len(jobs), 4):
                    quad = jobs[q0 : q0 + 4]
                    chains = []  # (slot, grp, job, cout0, m)
                    for ti, job in enumerate(quad):
                        for j, (c0, m) in enumerate(PARTS):
                            c = ti * 3 + j
                            chains.append((c // 4, c % 4, job, c0, m))
                    nslots = (len(chains) + 3) // 4
                    pss = [
                        psum.tile([128, PSUM_FREE], F32, name="ps", tag="ps")
                        for _ in range(nslots)
                    ]
                    kk = 0
                    for ci in range(2):
                        for dy in range(3):
                            for dx in range(3):
                                for slot, grp, (l, H, W, off, r0, nr), c0, m in chains:
                                    Wp = W + 1
                                    N = nr * Wp
                                    st = off + 1 + r0 * Wp + (dy - 1) * Wp + (dx - 1)
                                    nc.tensor.matmul(
                                        pss[slot][32 * grp : 32 * grp + m, 0:N],
                                        fwct[ci][:, dy * 3 + dx, c0 : c0 + m],
                                        src[ci][:, st : st + N],
                                        start=(kk == 0),
                                        stop=(kk == 17),
                                        tile_position=(0, 32 * grp),
                                    )
                                kk += 1
                    sgs = {}
                    for slot, grp, (l, H, W, off, r0, nr), c0, m in chains:
                        Wp = W + 1
                        N = nr * Wp
                        p0 = 32 * grp
                        if slot not in sgs:
                            sgs[slot] = stage.tile(
                                [128, PSUM_FREE], F32, name="sg", tag="sg"
                            )
                        sg = sgs[slot]
                        j = c0 // 32
                        ps_v = pss[slot][p0 : p0 + m, 0:N].rearrange(
                            "p (r w) -> p r w", w=Wp
                        )[:, :, 0:W]
                        nc.scalar.activation(
                            sg[p0 : p0 + m, 0 : nr * W].rearrange(
                                "p (r w) -> p r w", w=W
                            ),
                            ps_v,
                            AF.Identity,
                            bias=fbct[p0 : p0 + m, j : j + 1],
                        )
                        oc_flat = ocs[l].ap().rearrange("c h w -> c (h w)")
                        nc.sync.dma_start(
                            oc_flat[c0 : c0 + m, (r0 - 1) * W : (r0 - 1 + nr) * W],
                            sg[p0 : p0 + m, 0 : nr * W],
                        )

            def conv_final_rc(src, group):
                # 4 spatial tiles concurrently via PE column groups 0..3
                jobs = []
                for l, H, W, off in _segments(group):
                    for r0, nr in _level_tiles(H, W):
                        jobs.append((l, H, W, off, r0, nr))
                for q0 in range(0, len(jobs), 4):
                    quad = jobs[q0 : q0 + 4]
                    ps = psum.tile([128, PSUM_FREE], F32, name="ps", tag="ps")
                    kk = 0
                    for ci in range(2):
                        for dy in range(3):
                            for dx in range(3):
                                for g, (l, H, W, off, r0, nr) in enumerate(quad):
                                    Wp = W + 1
                                    N = nr * Wp
                                    st = (
                                        off + 1 + r0 * Wp + (dy - 1) * Wp + (dx - 1)
                                    )
                                    nc.tensor.matmul(
                                        ps[32 * g : 32 * g + 5, 0:N],
                                        fwrt[ci][:, dy * 3 + dx, :],
                                        src[ci][:, st : st + N],
                                        start=(kk == 0),
                                        stop=(kk == 17),
                                        tile_position=(0, 32 * g),
                                    )
                                kk += 1
                    sg = stage.tile([128, PSUM_FREE], F32, name="sg", tag="sg")
                    sg2 = stage.tile([128, PSUM_FREE], F32, name="sg2", tag="sg2")
                    for g, (l, H, W, off, r0, nr) in enumerate(quad):
                        Wp = W + 1
                        N = nr * Wp
                        p0 = 32 * g
                        or_flat = ors[l].ap().rearrange("c h w -> c (h w)")
                        ot_flat = ots[l].ap().rearrange("c h w -> c (h w)")
                        ps_v = ps[p0 : p0 + 5, 0:N].rearrange(
                            "p (r w) -> p r w", w=Wp
                        )[:, :, 0:W]
                        # y = scale*conv + bias (reg rows: coef*conv + coef*b;
                        # ctr row: conv + b)
                        nc.scalar.activation(
                            sg[p0 : p0 + 5, 0 : nr * W].rearrange(
                                "p (r w) -> p r w", w=W
                            ),
                            ps_v,
                            AF.Identity,
                            bias=fbrt[p0 : p0 + 5, l : l + 1],
                            scale=fsrt[p0 : p0 + 5, l : l + 1],
                        )
                        nc.scalar.activation(
                            sg2[p0 : p0 + 4, 0 : nr * W],
                            sg[p0 : p0 + 4, 0 : nr * W],
                            AF.Exp,
                        )
                        nc.sync.dma_start(
                            or_flat[:, (r0 - 1) * W : (r0 - 1 + nr) * W],
                            sg2[p0 : p0 + 4, 0 : nr * W],
                        )
                        nc.sync.dma_start(
                            ot_flat[:, (r0 - 1) * W : (r0 - 1 + nr) * W],
                            sg[p0 + 4 : p0 + 5, 0 : nr * W],
                        )

            # ---- first input load (head of the SP DMA ring) --------------
            memset_pads(bufP, SEG_GROUPS[0])
            load_x(bufP, SEG_GROUPS[0])
            memset_pads(bufQ, SEG_GROUPS[0])

            # ---- resident final weights / biases (SWDGE; off the SP ring)
            fwct = []
            fwrt = []
            for ci in range(2):
                w_tile = wpool.tile([128, 9, NCLS], BF16, name=f"fwc{ci}", tag=f"fwc{ci}")
                nc.gpsimd.dma_start(w_tile[:], fwc[ci, :, :, :])
                fwct.append(w_tile)
                w_tile = wpool.tile([128, 9, 5], BF16, name=f"fwr{ci}", tag=f"fwr{ci}")
                nc.gpsimd.dma_start(w_tile[:], fwr[ci, :, :, :])
                fwrt.append(w_tile)
            tbt = wpool.tile([128, 16], F32, name="tb", tag="tb")
            nc.gpsimd.dma_start(tbt[:], tb[:, :])
            fbct = wpool.tile([128, 3], F32, name="fbc", tag="fbc")
            nc.gpsimd.dma_start(fbct[:], fbc[:, :])
            fbrt = wpool.tile([128, 5], F32, name="fbr", tag="fbr")
            nc.gpsimd.dma_start(fbrt[:], fbr[:, :])
            fsrt = wpool.tile([128, 5], F32, name="fsr", tag="fsr")
            nc.gpsimd.dma_start(fsrt[:], fsr[:, :])

            # ---- main schedule: P/Q ping-pong, x re-loaded for reg tower -
            for gi, group in enumerate(SEG_GROUPS):
                conv_tower(bufP, bufQ, 0, 0, group)  # c1 cls (P holds x)
                conv_tower(bufQ, bufP, 0, 1, group)  # c2 cls
                conv_tower(bufP, bufQ, 0, 2, group)  # c3 cls
                conv_tower(bufQ, bufP, 0, 3, group)  # c4 cls
                conv_final_cls(bufP, group)
                load_x(bufQ, group)                  # re-load x (Q free after c4 cls)
                conv_tower(bufQ, bufP, 1, 0, group)  # c1 reg (P free after final cls)
                conv_tower(bufP, bufQ, 1, 1, group)  # c2 reg
                conv_tower(bufQ, bufP, 1, 2, group)  # c3 reg
                conv_tower(bufP, bufQ, 1, 3, group)  # c4 reg
                if gi + 1 < len(SEG_GROUPS):
                    # P free after c4 reg read it: prep next group's input
                    memset_pads(bufP, SEG_GROUPS[gi + 1])
                    load_x(bufP, SEG_GROUPS[gi + 1])
                conv_final_rc(bufQ, group)
                if gi + 1 < len(SEG_GROUPS):
                    memset_pads(bufQ, SEG_GROUPS[gi + 1])

    nc.compile()
    return nc


def _prep_weights(inputs):
    """Host-side: transpose conv weights into lhsT layouts, cast to bf16."""
    cls_w = np.asarray(inputs["cls_w"], np.float32)
    reg_w = np.asarray(inputs["reg_w"], np.float32)
    cls_b = np.asarray(inputs["cls_b"], np.float32)
    reg_b = np.asarray(inputs["reg_b"], np.float32)
    fcls_w = np.asarray(inputs["fcls_w"], np.float32)
    fcls_b = np.asarray(inputs["fcls_b"], np.float32)
    freg_w = np.asarray(inputs["freg_w"], np.float32)
    freg_b = np.asarray(inputs["freg_b"], np.float32)
    fctr_w = np.asarray(inputs["fctr_w"], np.float32)
    fctr_b = np.asarray(inputs["fctr_b"], np.float32)
    reg_coef = np.asarray(inputs["reg_coef"], np.float32)

    tw = np.empty((2, STACKED, 2, 128, 9 * 256), BF16_NP)
    for t, warr in enumerate((cls_w, reg_w)):
        for j in range(STACKED):
            # [O, I, 3, 3] -> [k, I, O]
            A = warr[j].transpose(2, 3, 1, 0).reshape(9, C, C)
            for ci in range(2):
                tw[t, j, ci] = (
                    A[:, ci * 128 : (ci + 1) * 128, :]
                    .transpose(1, 0, 2)
                    .reshape(128, 9 * 256)
                    .astype(BF16_NP)
                )

    def final_lhsT(w):  # [O, C, 3, 3] -> [2, 128, 9, O]
        O = w.shape[0]
        A = w.transpose(2, 3, 1, 0).reshape(9, C, O)
        out = np.empty((2, 128, 9, O), BF16_NP)
        for ci in range(2):
            out[ci] = A[:, ci * 128 : (ci + 1) * 128, :].transpose(1, 0, 2)
        return out

    fw_cls = final_lhsT(fcls_w)
    fw_rc = final_lhsT(np.concatenate([freg_w, fctr_w], axis=0))

    tb = np.zeros((128, 16), np.float32)
    for t, barr in enumerate((cls_b, reg_b)):
        for j in range(STACKED):
            for co in range(2):
                tb[:, (t * STACKED + j) * 2 + co] = barr[j, co * 128 : (co + 1) * 128]
    fb_cls = np.zeros((128, 3), np.float32)
    for g in range(4):
        fb_cls[32 * g : 32 * g + 32, 0] = fcls_b[0:32]
        fb_cls[32 * g : 32 * g + 32, 1] = fcls_b[32:64]
        fb_cls[32 * g : 32 * g + 16, 2] = fcls_b[64:80]
    fb_rc = np.zeros((128, 5), np.float32)
    fs_rc = np.ones((128, 5), np.float32)
    for l in range(len(LEVELS)):
        for g in range(4):
            fb_rc[32 * g : 32 * g + 4, l] = reg_coef[l] * freg_b
            fb_rc[32 * g + 4, l] = fctr_b[0]
            fs_rc[32 * g : 32 * g + 4, l] = reg_coef[l]
    return {
        "tw": tw,
        "fw_cls": fw_cls,
        "fw_rc": fw_rc,
        "tb": tb,
        "fb_cls": fb_cls,
        "fb_rc": fb_rc,
        "fs_rc": fs_rc,
    }


def kernel(**inputs):
    if "nc" not in _CACHE:
        _CACHE["nc"] = build_nc()
    nc = _CACHE["nc"]

    shared = _prep_weights(inputs)
    xs = [np.asarray(inputs[f"x{i}"], np.float32) for i in range(len(LEVELS))]
    xs_b = [x.astype(BF16_NP) for x in xs]

    in_maps = []
    for c in range(N_CORES):
        m = dict(shared)
        for i in range(len(LEVELS)):
            m[f"x{i}"] = np.ascontiguousarray(xs_b[i][c])
        in_maps.append(m)

    res = run_bass_kernel_spmd(nc, in_maps, core_ids=list(range(N_CORES)))

    cls_outs, reg_outs, ctr_outs = [], [], []
    for l in range(len(LEVELS)):
        cls_outs.append(np.stack([res.results[c][f"oc{l}"] for c in range(N_CORES)]))
        reg_outs.append(np.stack([res.results[c][f"or{l}"] for c in range(N_CORES)]))
        ctr_outs.append(np.stack([res.results[c][f"ot{l}"] for c in range(N_CORES)]))
    return tuple(cls_outs + reg_outs + ctr_outs)
